# revision 49
# baseline (speedup 1.0000x reference)
"""Trainium2 Bass kernel for nn_FeatureLossOursBMSE.

Model: s = conv1x1(preds_S) -> masked by checkerboard -> conv3x3 -> relu ->
conv3x3 = new_fea (t). Then pairwise Gram q[i,j] = <p_i, t_j> over D=C*H*W,
logits = -0.5*sq/64, ce = mean_i(logsumexp_j - diag), loss = ce*16*2e-5.
||p_i||^2 cancels exactly in (logsumexp_j logits[i,:] - logits[i,i]), so only
q[i,j] and ||t_j||^2 are needed:
  L[i,j] = q[i,j]/64 - ||t_j||^2/128;  ce_i = logsumexp_j L[i,j] - L[i,i].

Sharding: 8 cores, horizontal slab of 8 image rows per core, all 8 images.
Each core computes conv stack on its slab (with halo rows computed locally),
partial Gram q and ||t||^2 over its slab (D-sharded contraction), then one
72-float AllReduce and a replicated 8x8 softmax-CE tail.

Conv implementation: fp32r matmuls (full-rate on TRN2 for free>=256),
width-padded feature tiles [128, rows, 66] with zeroed border columns, taps
shift the rhs AP by kw in {0,1,2}; PSUM accumulates 18 matmuls per output
block. Checkerboard mask and align-conv bias are folded host-side: the mask
commutes with the 1x1 conv, and the bias rides an extra contraction row
whose input channel is the (row-validity-masked) mask itself.
"""

import numpy as np
from contextlib import ExitStack

import concourse.bass as bass
import concourse.mybir as mybir
import concourse.tile as tile
from concourse import bacc
from concourse.bass_utils import run_bass_kernel_spmd

F32 = mybir.dt.float32
F32R = mybir.dt.float32r
BF16 = mybir.dt.bfloat16
AF = mybir.ActivationFunctionType
ALU = mybir.AluOpType

N_CORES = 8
N, CS, CT, H, W = 8, 128, 256, 64, 64
RS = H // N_CORES  # slab rows per core = 8
NOISE_VAR = 64.0
ALPHA_MGD = 2e-05

LAST_RESULTS = None
_NC_CACHE = {}


def _build(repeats=1, use_collective=True, skip_gram=False, skip_conv=False, skip_amr=False, vtag_len=None, loop_n=None, tail_in_loop=False, tail_reps=None, psum_bufs=6, depth=2, dump=False):
    n_dev = N_CORES if use_collective else 1
    nc = bacc.Bacc("TRN2", target_bir_lowering=False, debug=False, num_devices=n_dev)
    # dummy input whose shape encodes the build variant: the remote compile
    # cache hashes HLO structure only (not backend_config), so structurally
    # identical variants would collide on the same cached NEFF
    if vtag_len is None:
        vtag_len = repeats
    vtag_in = nc.dram_tensor("vtag", [1, vtag_len], F32, kind="ExternalInput").ap()
    xs_in = nc.dram_tensor("xs", [N, 128, 12, 64], F32R, kind="ExternalInput").ap()
    xsm_in = nc.dram_tensor("xsm", [1, 12, 64], F32R, kind="ExternalInput").ap()
    cst_in = nc.dram_tensor("cst", [128, 262], F32R, kind="ExternalInput").ap()
    wam_in = nc.dram_tensor("wam", [1, 2, 128], F32R, kind="ExternalInput").ap()
    w1_in = nc.dram_tensor("w1", [128, 9, 2, 2, 128], F32R, kind="ExternalInput").ap()
    w2_in = nc.dram_tensor("w2", [128, 9, 2, 2, 128], F32R, kind="ExternalInput").ap()
    p_in = nc.dram_tensor("p", [128, N, 2, RS, 64], F32, kind="ExternalInput").ap()
    eye_in = nc.dram_tensor("eye8", [8, 8], F32, kind="ExternalInput").ap()
    loss_out = nc.dram_tensor("loss", [1, 1], F32, kind="ExternalOutput").ap()
    if dump:
        t_dbg = nc.dram_tensor("t_dbg", [N, 128, 2, RS, 64], F32, kind="ExternalOutput").ap()
        q_dbg = nc.dram_tensor("q_dbg", [1, 72], F32, kind="ExternalOutput").ap()
        mk_dbg = nc.dram_tensor("mk_dbg", [2, 128, 12, 66], F32, kind="ExternalOutput").ap()
        r1_dbg = nc.dram_tensor("r1_dbg", [2, 128, 10, 66], F32, kind="ExternalOutput").ap()

    with tile.TileContext(nc) as tc:
        with ExitStack() as ctx:
            wpool = ctx.enter_context(tc.tile_pool(name="weights", bufs=1))
            xpool = ctx.enter_context(tc.tile_pool(name="xs", bufs=3))
            fpool = ctx.enter_context(tc.tile_pool(name="feat", bufs=1))
            ppool = ctx.enter_context(tc.tile_pool(name="p", bufs=1))
            spool = ctx.enter_context(tc.tile_pool(name="scratch", bufs=2))
            cpool = ctx.enter_context(tc.tile_pool(name="ce", bufs=1))
            psum = ctx.enter_context(tc.tile_pool(name="psum", bufs=psum_bufs, space="PSUM"))
            cps = ctx.enter_context(tc.tile_pool(name="ceps", bufs=1, space="PSUM"))
            dram = ctx.enter_context(tc.tile_pool(name="dram", bufs=1, space="DRAM"))

            # ---- weights / constants (one blob DMA) ----
            cst_t = wpool.tile([128, 262], F32R)
            wam_t = wpool.tile([1, 2, 128], F32R)
            w1_t = wpool.tile([128, 9, 2, 2, 128], F32R)
            w2_t = wpool.tile([128, 9, 2, 2, 128], F32R)
            eye_t = wpool.tile([8, 8], F32)
            xsm_t = wpool.tile([1, 12, 64], F32R)
            nc.sync.dma_start(out=cst_t[:], in_=cst_in)
            wa_t = cst_t[:, 0:256].rearrange("p (a b) -> p a b", a=2)
            b1_t = cst_t[:, 256:258].bitcast(F32)
            b2_t = cst_t[:, 258:260].bitcast(F32)
            rm_t = cst_t[:, 260:262].bitcast(F32)
            nc.sync.dma_start(out=wam_t[:], in_=wam_in)
            nc.sync.dma_start(out=eye_t[:], in_=eye_in)
            nc.sync.dma_start(out=xsm_t[:], in_=xsm_in)

            p_t = ppool.tile([128, N, 2, RS, 64], F32)

            def _dma_w1():
                nc.sync.dma_start(out=w1_t[:], in_=w1_in)

            def _dma_w2p():
                nc.sync.dma_start(out=w2_t[:], in_=w2_in)
                nc.sync.dma_start(out=p_t[:], in_=p_in)

            def _emit_big_dmas():
                _dma_w1()
                _dma_w2p()

            ones = cpool.tile([128, 1], F32)
            nc.vector.memset(ones[:], 1.0)
            ones_row = cpool.tile([1, 8], F32)
            nc.vector.memset(ones_row[:], 1.0)

            # ---- persistent feature tiles (border cols pre-zeroed) ----
            masked = [
                [
                    fpool.tile([128, 12, 66], F32R, name=f"mk{par}_{oi}", tag=f"mk{par}_{oi}")
                    for oi in range(2)
                ]
                for par in range(depth)
            ]
            relu1 = [
                [
                    fpool.tile([128, 10, 66], F32R, name=f"r1{par}_{oi}", tag=f"r1{par}_{oi}")
                    for oi in range(2)
                ]
                for par in range(depth)
            ]
            t_tiles = [
                fpool.tile([128, 2, RS, 64], F32, name=f"t{n}", tag=f"t{n}")
                for n in range(N)
            ]
            zcol = cpool.tile([128, 12], F32)
            nc.vector.memset(zcol[:], 0.0)
            for par in range(depth):
                for oi in range(2):
                    nc.scalar.copy(masked[par][oi][:, :, 0:1], zcol[:, 0:12, None])
                    nc.scalar.copy(masked[par][oi][:, :, 65:66], zcol[:, 0:12, None])
                    nc.scalar.copy(relu1[par][oi][:, :, 0:1], zcol[:, 0:10, None])
                    nc.scalar.copy(relu1[par][oi][:, :, 65:66], zcol[:, 0:10, None])

            # gram accumulators: acc[:, n*8+i], tnorm[:, n]
            acc = cpool.tile([128, 64], F32)
            tnorm = cpool.tile([128, 8], F32)
            if skip_gram or skip_amr or skip_conv:
                nc.vector.memset(acc[:], 0.0)
                nc.vector.memset(tnorm[:], 1.0)
            part = cps.tile([1, 72], F32, name="part", tag="cepart")

            # ---- per-image stages ----
            def conv1x1(n):
                xs_t = xpool.tile([128, 12, 64], F32R, name="xs_t", tag="xs_t")
                nc.sync.dma_start(out=xs_t[:], in_=xs_in[n])
                mk = masked[n % depth]
                for oi in range(2):
                    for r0, R in ((0, 8), (8, 4)):
                        ps = psum.tile([128, 512], F32, name="ps_c1", tag="ps")
                        ps3 = ps[:, 0 : R * 64].rearrange("p (r c) -> p r c", r=R)
                        nc.tensor.matmul(
                            ps3, wa_t[:, oi, :], xs_t[:, r0 : r0 + R, :],
                            start=True, stop=False,
                        )
                        nc.tensor.matmul(
                            ps3, wam_t[:, oi, :], xsm_t[:, r0 : r0 + R, :],
                            start=False, stop=True,
                        )
                        nc.scalar.copy(mk[oi][:, r0 : r0 + R, 1:65], ps3)

            def conv3x3(w_t, src, b_t, n, rows_out, fb, dst_fn, func):
                # dst_fn(oi) -> (out_ap rows x 64). src: 2-chunk padded tiles.
                for oi in range(2):
                    for r0, R in fb:
                        ps = psum.tile([128, 512], F32, name="ps_g", tag="ps")
                        ps3 = ps[:, 0 : R * 64].rearrange("p (r c) -> p r c", r=R)
                        k = 0
                        for t in range(9):
                            kh, kw = t // 3, t % 3
                            for ci in range(2):
                                nc.tensor.matmul(
                                    ps3,
                                    w_t[:, t, ci, oi, :],
                                    src[ci][:, r0 + kh : r0 + kh + R, kw : kw + 64],
                                    start=(k == 0),
                                    stop=(k == 17),
                                )
                                k += 1
                        nc.scalar.activation(
                            dst_fn(oi, r0, R), ps3, func, bias=b_t[:, oi : oi + 1],
                        )

            def gen1(n):
                mk, r1 = masked[n % depth], relu1[n % depth]
                conv3x3(
                    w1_t, mk, b1_t, n, 10, ((0, 5), (5, 5)),
                    lambda oi, r0, R: r1[oi][:, r0 : r0 + R, 1:65],
                    AF.Relu,
                )
                # zero out-of-image halo rows (rows 0 and 9) via rowmask
                for oi in range(2):
                    hal = r1[oi][:, 0:10:9, :]
                    nc.vector.tensor_tensor(
                        out=hal, in0=hal,
                        in1=rm_t[:, :, None].to_broadcast([128, 2, 66]),
                        op=ALU.mult,
                    )

            def gen2(n):
                r1 = relu1[n % depth]
                conv3x3(
                    w2_t, r1, b2_t, n, RS, ((0, 4), (4, 4)),
                    lambda oi, r0, R: t_tiles[n][:, oi, r0 : r0 + R, :],
                    AF.Identity,
                )

            def gram(n, split=False):
                tt = t_tiles[n]
                sq = spool.tile([128, 1024], F32, name="sq_s", tag="sq_s")
                nc.scalar.activation(
                    sq[:].rearrange("p (a b) -> p a b", a=2), tt[:],
                    AF.Square, accum_out=tnorm[:, n : n + 1],
                )
                if skip_amr:
                    return
                if not split:
                    for i in range(N):
                        gs = spool.tile([128, 1024], F32, name="gr_s", tag="gr_s")
                        col = i * 8 + n
                        nc.vector.affine_mul_reduce(
                            out=gs[:].rearrange("p (a b) -> p a b", a=2),
                            accum_out=acc[:, col : col + 1],
                            in0=p_t[:, i].rearrange("p a r c -> p a (r c)"),
                            in1=tt[:].rearrange("p a r c -> p a (r c)"),
                            scale=1.0,
                            bias=0.0,
                        )
                    return
                # split: one AMR per (quarter, i); partial sums in acc4, then
                # a 4->1 add chain per i on the vector engine
                acc4 = cpool.tile([128, 8, 4], F32, name="acc4", tag="acc4")
                for q in range(4):
                    oi, r0 = q // 2, (q % 2) * 4
                    for i in range(N):
                        gs = spool.tile([128, 1024], F32, name="gr_s", tag="gr_s")
                        nc.vector.affine_mul_reduce(
                            out=gs[:, 0:256].rearrange("p (r c) -> p r c", r=4),
                            accum_out=acc4[:, i, q : q + 1],
                            in0=p_t[:, i, oi, r0 : r0 + 4],
                            in1=tt[:, oi, r0 : r0 + 4],
                            scale=1.0,
                            bias=0.0,
                        )
                a2 = cpool.tile([128, 8, 2], F32, name="acc2q", tag="acc2q")
                nc.vector.tensor_add(a2[:], acc4[:, :, 0:2], acc4[:, :, 2:4])
                accv = acc[:].rearrange("p (i j) -> p i j", i=8)
                nc.vector.tensor_add(accv[:, :, n], a2[:, :, 0], a2[:, :, 1])


            def _ce_tail():
                # ---- CE tail (replicated on every core) ----
                nc.tensor.matmul(part[:, 0:64], ones[:], acc[:], start=True, stop=False)
                nc.tensor.matmul(part[:, 64:72], ones[:], tnorm[:], start=False, stop=True)
                part_sb = cpool.tile([1, 72], F32)
                nc.scalar.copy(part_sb[:], part[:])
                if dump:
                    nc.sync.dma_start(out=q_dbg, in_=part_sb[:])
                    for n in range(N):
                        nc.sync.dma_start(out=t_dbg[n], in_=t_tiles[n][:])
                    for oi in range(2):
                        nc.sync.dma_start(out=mk_dbg[oi], in_=masked[0][oi][:].bitcast(F32))
                        nc.sync.dma_start(out=r1_dbg[oi], in_=relu1[0][oi][:].bitcast(F32))

                cc_in = dram.tile([1, 72], F32)
                cc_out = dram.tile([1, 72], F32)
                nc.sync.dma_start(out=cc_in[:], in_=part_sb[:])
                if use_collective:
                    nc.gpsimd.collective_compute(
                        "AllReduce", ALU.add, replica_groups=[list(range(N_CORES))],
                        ins=[cc_in[:].opt()], outs=[cc_out[:].opt()],
                    )
                else:
                    nc.gpsimd.dma_start(out=cc_out[:], in_=cc_in[:])

                q_sb = cpool.tile([8, 8], F32)
                tn_row = cpool.tile([1, 8], F32)
                nc.sync.dma_start(
                    out=q_sb[:], in_=cc_out[:, 0:64].rearrange("a (i j) -> (a i) j", i=8)
                )
                nc.sync.dma_start(out=tn_row[:], in_=cc_out[:, 64:72])

                # L[i,j] = q/64 - tn[j]/128 via PE broadcast
                q_s = cpool.tile([8, 8], F32)
                nc.scalar.mul(q_s[:], q_sb[:], 1.0 / NOISE_VAR)
                tn_neg = cpool.tile([1, 8], F32)
                nc.scalar.mul(tn_neg[:], tn_row[:], -1.0 / (2.0 * NOISE_VAR))
                L_ps = cps.tile([8, 8], F32, name="L_ps", tag="ceps")
                nc.tensor.matmul(L_ps[:], eye_t[:], q_s[:], start=True, stop=False)
                nc.tensor.matmul(L_ps[:], ones_row[:], tn_neg[:], start=False, stop=True)
                L = cpool.tile([8, 8], F32)
                nc.vector.tensor_copy(L[:], L_ps[:])

                m = cpool.tile([8, 1], F32)
                nc.vector.reduce_max(m[:], L[:], axis=mybir.AxisListType.X)
                negm = cpool.tile([8, 1], F32)
                nc.scalar.mul(negm[:], m[:], -1.0)
                e = cpool.tile([8, 8], F32)
                nc.scalar.activation(e[:], L[:], AF.Exp, bias=negm[:, 0:1], scale=1.0)
                s = cpool.tile([8, 1], F32)
                nc.vector.reduce_sum(s[:], e[:], axis=mybir.AxisListType.X)
                ln_s = cpool.tile([8, 1], F32)
                nc.scalar.activation(ln_s[:], s[:], AF.Ln)

                # diag(L) via eye mask (avoids strided DRAM readbacks)
                ldm = cpool.tile([8, 8], F32)
                nc.vector.tensor_mul(ldm[:], L[:], eye_t[:])
                ld = cpool.tile([8, 1], F32)
                nc.vector.reduce_sum(ld[:], ldm[:], axis=mybir.AxisListType.X)
                ce = cpool.tile([8, 1], F32)
                nc.vector.tensor_add(ce[:], m[:], ln_s[:])
                nc.vector.tensor_sub(ce[:], ce[:], ld[:])

                lp = cps.tile([1, 1], F32, name="lp", tag="ceps")
                nc.tensor.matmul(lp[:], ce[:], ones[0:8, :], start=True, stop=True)
                l_sb = cpool.tile([1, 1], F32)
                # loss = sum_i ce_i / N * (2*NOISE_VAR/N) * ALPHA
                nc.scalar.mul(l_sb[:], lp[:], (2.0 * NOISE_VAR / (N * N)) * ALPHA_MGD)
                nc.sync.dma_start(out=loss_out, in_=l_sb[:])


            if skip_conv:
                for n in range(N):
                    nc.vector.memset(t_tiles[n][:], 0.125)

            def _body_once(stage_dmas=False):
                if skip_conv:
                    if stage_dmas:
                        _emit_big_dmas()
                    for n in range(N):
                        gram(n)
                else:
                    conv1x1(0)
                    if stage_dmas:
                        _dma_w1()
                    for k in range(1, depth):
                        conv1x1(k)
                    for n in range(N):
                        gen1(n)
                        if stage_dmas and n == 0:
                            _dma_w2p()
                        gen2(n)
                        if n + depth < N:
                            conv1x1(n + depth)
                        if not skip_gram:
                            gram(n, split=(n == N - 1))

            if tail_reps is not None:
                nc.vector.memset(acc[:], 0.0)
                nc.vector.memset(tnorm[:], 1.0)
                for _ in range(tail_reps):
                    _ce_tail()
            elif loop_n is not None:
                _emit_big_dmas()
                if tail_in_loop:
                    with tc.For_i(0, loop_n, 1) as _i:
                        _body_once()
                        _ce_tail()
                else:
                    with tc.For_i(0, loop_n, 1) as _i:
                        _body_once()
                    _ce_tail()
            else:
                for _rep in range(repeats):
                    _body_once(stage_dmas=(_rep == 0))
                    _ce_tail()

    nc.compile()
    return nc


def _build_v2(repeats=1, use_collective=True, vtag_len=None, loop_n=None,
              reps_per_iter=1, tail_in_loop=False, tail_reps=None, psum_bufs=6,
              dump=False, skip_gram=False, skip_conv=False, skip_amr=False,
              bf16_gram=False):
    """Parity-optimized build.

    The checkerboard mask zeroes even sites of s, so (a) conv1x1 computes
    only odd-site outputs (half the rows), stored column-deinterleaved as
    M[p, img, row, slot] with slot j <-> padded col cc = 2j + (row%2); and
    (b) gen1 skips taps whose input site is even: for an output block of
    fixed (row-parity rho, col-parity gam), only taps with
    (kh+kw)%2 == (rho+gam)%2 contribute (4 or 5 of 9). Halves gen1 PE work
    and instruction count. Blocks span an image PAIR to keep the fp32r
    full-rate free>=256 threshold. gen2 is dense and unchanged
    arithmetically, but also paired (free 512).
    """
    n_dev = N_CORES if use_collective else 1
    nc = bacc.Bacc("TRN2", target_bir_lowering=False, debug=False, num_devices=n_dev)
    if vtag_len is None:
        vtag_len = repeats
    vtag_in = nc.dram_tensor("vtag", [1, vtag_len], F32, kind="ExternalInput").ap()
    xs_in = nc.dram_tensor("xs", [N, 128, 12, 64], F32R, kind="ExternalInput").ap()
    cst_in = nc.dram_tensor("cst", [128, 268], F32R, kind="ExternalInput").ap()
    w1_in = nc.dram_tensor("w1", [128, 9, 2, 2, 128], F32R, kind="ExternalInput").ap()
    w2_in = nc.dram_tensor("w2", [128, 9, 2, 2, 128], F32R, kind="ExternalInput").ap()
    GDT = BF16 if bf16_gram else F32
    p_in = nc.dram_tensor("p", [128, N, 2, RS, 64], GDT, kind="ExternalInput").ap()
    eye_in = nc.dram_tensor("eye8", [8, 8], F32, kind="ExternalInput").ap()
    loss_out = nc.dram_tensor("loss", [1, 1], F32, kind="ExternalOutput").ap()
    if dump:
        m_dbg = nc.dram_tensor("m_dbg", [2, 128, 2, 12, 33], F32, kind="ExternalOutput").ap()
        r1_dbg = nc.dram_tensor("r1_dbg", [2, 128, 2, 10, 66], F32, kind="ExternalOutput").ap()
        t_dbg = nc.dram_tensor("t_dbg", [N, 128, 2, RS, 64], F32, kind="ExternalOutput").ap()
        q_dbg = nc.dram_tensor("q_dbg", [1, 72], F32, kind="ExternalOutput").ap()

    TAPS = {par: [(kh, kw) for kh in range(3) for kw in range(3)
                  if (kh + kw) % 2 == par] for par in (0, 1)}

    with tile.TileContext(nc) as tc:
        with ExitStack() as ctx:
            wpool = ctx.enter_context(tc.tile_pool(name="weights", bufs=1))
            xpool = ctx.enter_context(tc.tile_pool(name="xs", bufs=4))
            fpool = ctx.enter_context(tc.tile_pool(name="feat", bufs=1))
            ppool = ctx.enter_context(tc.tile_pool(name="p", bufs=1))
            spool = ctx.enter_context(tc.tile_pool(name="scratch", bufs=2))
            qpool = ctx.enter_context(tc.tile_pool(name="qscratch", bufs=2))
            cpool = ctx.enter_context(tc.tile_pool(name="ce", bufs=1))
            # PSUM banks: 3 (conv1x1+gen1, 512 f32) + 2x2 (gen2, 1024 f32)
            # + 1 (ce tail) = 8
            psum = ctx.enter_context(tc.tile_pool(name="psum", bufs=3, space="PSUM"))
            gpsum = ctx.enter_context(tc.tile_pool(name="gpsum", bufs=2, space="PSUM"))
            cps = ctx.enter_context(tc.tile_pool(name="ceps", bufs=1, space="PSUM"))
            dram = ctx.enter_context(tc.tile_pool(name="dram", bufs=1, space="DRAM"))

            # ---- weights / constants ----
            cst_t = wpool.tile([128, 268], F32R)
            w1_t = wpool.tile([128, 9, 2, 2, 128], F32R)
            w2_t = wpool.tile([128, 9, 2, 2, 128], F32R)
            eye_t = wpool.tile([8, 8], F32)
            nc.sync.dma_start(out=cst_t[:], in_=cst_in)
            wa_t = cst_t[:, 0:256].rearrange("p (a b) -> p a b", a=2)
            b1_t = cst_t[:, 256:258].bitcast(F32)
            b2_t = cst_t[:, 258:260].bitcast(F32)
            rm_t = cst_t[:, 260:262].bitcast(F32)
            ba_t = cst_t[:, 262:264].bitcast(F32)
            hm_t = cst_t[:, 264:268].bitcast(F32)
            nc.sync.dma_start(out=eye_t[:], in_=eye_in)

            p_t = ppool.tile([128, N, 2, RS, 64], GDT)

            def _emit_big_dmas():
                nc.sync.dma_start(out=w1_t[:], in_=w1_in)
                nc.sync.dma_start(out=w2_t[:], in_=w2_in)
                nc.sync.dma_start(out=p_t[:], in_=p_in)

            ones = cpool.tile([128, 1], F32)
            nc.vector.memset(ones[:], 1.0)
            ones_row = cpool.tile([1, 8], F32)
            nc.vector.memset(ones_row[:], 1.0)

            # ---- persistent feature tiles ----
            Ms = [[fpool.tile([128, 2, 12, 33], F32R, name=f"M{d}_{oi}", tag=f"M{d}_{oi}")
                   for oi in range(2)] for d in range(2)]
            R1s = [[fpool.tile([128, 2, 10, 66], F32R, name=f"R{d}_{oi}", tag=f"R{d}_{oi}")
                    for oi in range(2)] for d in range(2)]
            Ts = [fpool.tile([128, 2, 2, RS, 64], GDT, name=f"T{g}", tag=f"T{g}")
                  for g in range(4)]
            zf = cpool.tile([128, 1320], F32)
            nc.vector.memset(zf[:], 0.0)
            for d in range(2):
                for oi in range(2):
                    nc.scalar.copy(
                        Ms[d][oi][:].rearrange("p u r j -> p (u r j)"), zf[:, 0:792])
                    nc.scalar.copy(
                        R1s[d][oi][:].rearrange("p u r j -> p (u r j)"), zf[:, 0:1320])

            acc = cpool.tile([128, 64], F32)
            tnorm = cpool.tile([128, 8], F32)
            part = cps.tile([1, 72], F32, name="part", tag="ceps")

            # ---- stages ----
            def load_xs(g):
                XS = xpool.tile([128, 2, 12, 64], F32R, name="xs_t", tag="xs_t")
                nc.sync.dma_start(out=XS[:, 0], in_=xs_in[2 * g])
                nc.sync.dma_start(out=XS[:, 1], in_=xs_in[2 * g + 1])
                return XS

            def conv1x1(g, XS):
                d = g % 2
                for oi in range(2):
                    for rho in range(2):
                        ps = psum.tile([128, 512], F32, name="ps_c1", tag="ps")
                        psv = ps[:, 0:384].rearrange("p (u r c) -> p u r c", u=2, r=6)
                        nc.tensor.matmul(
                            psv, wa_t[:, oi, :],
                            XS[:, :, rho:12:2, (1 - rho):64:2],
                            start=True, stop=True,
                        )
                        j0 = 1 - rho
                        # per-partition b_align bias rides the ACT drain; the
                        # (core 0/7 only) out-of-image halo rows get bias too
                        # and are re-zeroed below on Pool
                        nc.scalar.activation(
                            Ms[d][oi][:, :, rho:12:2, j0:j0 + 32], psv,
                            AF.Identity, bias=ba_t[:, oi:oi + 1],
                        )
                    for r0, h0 in ((0, 0), (10, 2)):
                        hal = Ms[d][oi][:, :, r0:r0 + 2, :]
                        nc.gpsimd.tensor_tensor(
                            out=hal, in0=hal,
                            in1=hm_t[:, None, h0:h0 + 2, None]
                                .to_broadcast([128, 2, 2, 33]),
                            op=ALU.mult,
                        )

            def gen1(g):
                d = g % 2
                for oi in range(2):
                    for rho in range(2):
                        for gam in range(2):
                            taps = TAPS[(rho + gam) % 2]
                            ps = psum.tile([128, 512], F32, name="ps_g1", tag="ps")
                            psv = ps[:, 0:320].rearrange("p (u r c) -> p u r c", u=2, r=5)
                            k, last = 0, len(taps) * 2 - 1
                            for kh, kw in taps:
                                j0 = (gam + kw - ((rho + kh) % 2)) // 2
                                for ci in range(2):
                                    nc.tensor.matmul(
                                        psv, w1_t[:, kh * 3 + kw, ci, oi, :],
                                        Ms[d][ci][:, :, rho + kh:rho + kh + 9:2, j0:j0 + 32],
                                        start=(k == 0), stop=(k == last),
                                    )
                                    k += 1
                            nc.scalar.activation(
                                R1s[d][oi][:, :, rho:10:2, gam + 1:gam + 65:2],
                                psv, AF.Relu, bias=b1_t[:, oi:oi + 1],
                            )
                    hal = R1s[d][oi][:, :, 0:10:9, :]
                    nc.gpsimd.tensor_tensor(
                        out=hal, in0=hal,
                        in1=rm_t[:, None, :, None].to_broadcast([128, 2, 2, 66]),
                        op=ALU.mult,
                    )

            def gen2(g):
                d = g % 2
                for oi in range(2):
                    # one [128,1024] psum = 2 banks; block b fills bank b
                    ps = gpsum.tile([128, 1024], F32, name="ps_g2", tag="gps")
                    ps5 = ps[:].rearrange("p (b u r c) -> p b u r c", b=2, u=2, r=4)
                    for b in range(2):
                        k = 0
                        for t9 in range(9):
                            kh, kw = t9 // 3, t9 % 3
                            for ci in range(2):
                                nc.tensor.matmul(
                                    ps5[:, b], w2_t[:, t9, ci, oi, :],
                                    R1s[d][ci][:, :, b * 4 + kh:b * 4 + kh + 4, kw:kw + 64],
                                    start=(k == 0), stop=(k == 17),
                                )
                                k += 1
                    for u in range(2):
                        nc.scalar.activation(
                            Ts[g][:, u, oi], ps5[:, :, u], AF.Identity,
                            bias=b2_t[:, oi:oi + 1],
                        )

            def gram(n, split=False):
                tt = Ts[n // 2][:, n % 2]
                sq = spool.tile([128, 1024], F32, name="sq_s", tag="sq_s")
                nc.scalar.activation(
                    sq[:].rearrange("p (a b) -> p a b", a=2), tt,
                    AF.Square, accum_out=tnorm[:, n:n + 1],
                )
                if skip_amr:
                    return
                if not split:
                    for i in range(N):
                        gs = spool.tile([128, 1024], GDT, name="gr_s", tag="gr_s")
                        col = i * 8 + n
                        nc.vector.affine_mul_reduce(
                            out=gs[:].rearrange("p (a b) -> p a b", a=2),
                            accum_out=acc[:, col:col + 1],
                            in0=p_t[:, i].rearrange("p a r c -> p a (r c)"),
                            in1=tt.rearrange("p a r c -> p a (r c)"),
                            scale=1.0, bias=0.0,
                        )
                    return
                # split per oi half so the oi=0 dots start before gen2's
                # oi=1 matmuls finish (shortens the end-of-body DVE drain)
                a2 = cpool.tile([128, 8, 2], F32, name="acc2h", tag="acc2h")
                for oi in range(2):
                    for i in range(N):
                        gs = spool.tile([128, 1024], GDT, name="gr_s", tag="gr_s")
                        nc.vector.affine_mul_reduce(
                            out=gs[:, 0:512].rearrange("p (r c) -> p r c", r=RS),
                            accum_out=a2[:, i, oi:oi + 1],
                            in0=p_t[:, i, oi],
                            in1=tt[:, oi],
                            scale=1.0, bias=0.0,
                        )
                accv = acc[:].rearrange("p (i j) -> p i j", i=8)
                nc.vector.tensor_add(accv[:, :, n], a2[:, :, 0], a2[:, :, 1])

            def _ce_tail():
                nc.tensor.matmul(part[:, 0:64], ones[:], acc[:], start=True, stop=False)
                nc.tensor.matmul(part[:, 64:72], ones[:], tnorm[:], start=False, stop=True)
                part_sb = cpool.tile([1, 72], F32)
                nc.scalar.copy(part_sb[:], part[:])
                if dump:
                    nc.sync.dma_start(out=q_dbg, in_=part_sb[:])
                    for n in range(N):
                        nc.sync.dma_start(out=t_dbg[n], in_=Ts[n // 2][:, n % 2])
                    for oi in range(2):
                        nc.sync.dma_start(out=m_dbg[oi], in_=Ms[0][oi][:].bitcast(F32))
                        nc.sync.dma_start(out=r1_dbg[oi], in_=R1s[0][oi][:].bitcast(F32))

                cc_in = dram.tile([1, 72], F32)
                cc_out = dram.tile([1, 72], F32)
                nc.sync.dma_start(out=cc_in[:], in_=part_sb[:])
                if use_collective:
                    nc.gpsimd.collective_compute(
                        "AllReduce", ALU.add, replica_groups=[list(range(N_CORES))],
                        ins=[cc_in[:].opt()], outs=[cc_out[:].opt()],
                    )
                else:
                    nc.gpsimd.dma_start(out=cc_out[:], in_=cc_in[:])

                q_sb = cpool.tile([8, 8], F32)
                tn_row = cpool.tile([1, 8], F32)
                nc.sync.dma_start(
                    out=q_sb[:], in_=cc_out[:, 0:64].rearrange("a (i j) -> (a i) j", i=8)
                )
                nc.sync.dma_start(out=tn_row[:], in_=cc_out[:, 64:72])

                q_s = cpool.tile([8, 8], F32)
                nc.scalar.mul(q_s[:], q_sb[:], 1.0 / NOISE_VAR)
                tn_neg = cpool.tile([1, 8], F32)
                nc.scalar.mul(tn_neg[:], tn_row[:], -1.0 / (2.0 * NOISE_VAR))
                L_ps = cps.tile([8, 8], F32, name="L_ps", tag="ceps")
                nc.tensor.matmul(L_ps[:], eye_t[:], q_s[:], start=True, stop=False)
                nc.tensor.matmul(L_ps[:], ones_row[:], tn_neg[:], start=False, stop=True)
                L = cpool.tile([8, 8], F32)
                nc.vector.tensor_copy(L[:], L_ps[:])

                m = cpool.tile([8, 1], F32)
                nc.vector.reduce_max(m[:], L[:], axis=mybir.AxisListType.X)
                negm = cpool.tile([8, 1], F32)
                nc.scalar.mul(negm[:], m[:], -1.0)
                e = cpool.tile([8, 8], F32)
                nc.scalar.activation(e[:], L[:], AF.Exp, bias=negm[:, 0:1], scale=1.0)
                s = cpool.tile([8, 1], F32)
                nc.vector.reduce_sum(s[:], e[:], axis=mybir.AxisListType.X)
                ln_s = cpool.tile([8, 1], F32)
                nc.scalar.activation(ln_s[:], s[:], AF.Ln)

                ldm = cpool.tile([8, 8], F32)
                nc.vector.tensor_mul(ldm[:], L[:], eye_t[:])
                ld = cpool.tile([8, 1], F32)
                nc.vector.reduce_sum(ld[:], ldm[:], axis=mybir.AxisListType.X)
                ce = cpool.tile([8, 1], F32)
                nc.vector.tensor_add(ce[:], m[:], ln_s[:])
                nc.vector.tensor_sub(ce[:], ce[:], ld[:])

                lp = cps.tile([1, 1], F32, name="lp", tag="ceps")
                nc.tensor.matmul(lp[:], ce[:], ones[0:8, :], start=True, stop=True)
                l_sb = cpool.tile([1, 1], F32)
                nc.scalar.mul(l_sb[:], lp[:], (2.0 * NOISE_VAR / (N * N)) * ALPHA_MGD)
                nc.sync.dma_start(out=loss_out, in_=l_sb[:])

            if skip_gram or skip_conv or skip_amr:
                nc.vector.memset(acc[:], 0.0)
                nc.vector.memset(tnorm[:], 1.0)
            if skip_conv:
                for g in range(4):
                    nc.vector.memset(Ts[g][:], 0.125)

            def _body_once():
                if skip_conv:
                    for n in range(N):
                        gram(n)
                    return
                xs_tiles = [load_xs(g) for g in range(4)]
                conv1x1(0, xs_tiles[0])
                for g in range(4):
                    gen1(g)
                    if g + 1 < 4:
                        conv1x1(g + 1, xs_tiles[g + 1])
                    gen2(g)
                    if not skip_gram:
                        gram(2 * g, split=(g == 3))
                        gram(2 * g + 1, split=(g == 3))

            _emit_big_dmas()
            if tail_reps is not None:
                nc.vector.memset(acc[:], 0.0)
                nc.vector.memset(tnorm[:], 1.0)
                for _ in range(tail_reps):
                    _ce_tail()
            elif loop_n is not None:
                if tail_in_loop:
                    with tc.For_i(0, loop_n, 1) as _i:
                        _body_once()
                        _ce_tail()
                else:
                    with tc.For_i(0, loop_n, 1) as _i:
                        for _r in range(reps_per_iter):
                            _body_once()
                    _ce_tail()
            else:
                for _rep in range(repeats):
                    _body_once()
                    _ce_tail()

    nc.compile()
    return nc


def _prep_inputs_v2(preds_S, preds_T, W_align, b_align, W_gen1, b_gen1, W_gen2,
                    b_gen2, bf16_gram=False):
    f32 = np.float32
    pdt = mybir.dt.np(BF16) if bf16_gram else f32

    wa = np.ascontiguousarray(W_align[:, :, 0, 0].T.reshape(128, 256), f32)
    ba = np.ascontiguousarray(np.asarray(b_align, f32).reshape(2, 128).T, f32)

    def pack_w(Wg):
        w = Wg.reshape(2, 128, 2, 128, 3, 3)
        w = w.transpose(3, 4, 5, 2, 0, 1)
        return np.ascontiguousarray(w.reshape(128, 9, 2, 2, 128), f32)

    w1 = pack_w(np.asarray(W_gen1, f32))
    w2 = pack_w(np.asarray(W_gen2, f32))
    b1 = np.ascontiguousarray(b_gen1.reshape(2, 128).T, f32)
    b2 = np.ascontiguousarray(b_gen2.reshape(2, 128).T, f32)
    eye8 = np.eye(8, dtype=f32)

    in_maps = []
    for c in range(N_CORES):
        rows = np.arange(8 * c - 2, 8 * c + 10)
        valid = (rows >= 0) & (rows < H)
        vr = rows[valid]
        xs = np.zeros((N, 128, 12, 64), f32)
        xs[:, :, valid] = preds_S[:, :, vr, :]
        rm = np.broadcast_to(
            np.array([1.0 if c > 0 else 0.0, 1.0 if c < 7 else 0.0], f32), (128, 2)
        )
        # halo-row validity for M rows 0,1,10,11 (conv1x1 bias fixup)
        hm = np.broadcast_to(
            valid[[0, 1, 10, 11]].astype(f32), (128, 4)
        )
        cst = np.concatenate([wa, b1, b2, rm, ba, hm], axis=1).astype(f32)
        slab = preds_T[:, :, 8 * c: 8 * c + RS, :].reshape(N, 2, 128, RS, 64)
        p = np.ascontiguousarray(slab.transpose(2, 0, 1, 3, 4)).astype(pdt)
        in_maps.append(
            {
                "xs": xs, "cst": cst, "w1": w1, "w2": w2,
                "p": p, "eye8": eye8,
            }
        )
    return in_maps


def _prep_inputs(preds_S, preds_T, W_align, b_align, W_gen1, b_gen1, W_gen2, b_gen2):
    f32 = np.float32
    mat = ((np.arange(H)[:, None] + np.arange(W)[None, :]) % 2).astype(f32)

    wa = np.ascontiguousarray(W_align[:, :, 0, 0].T.reshape(128, 256), f32)
    wam = np.ascontiguousarray(b_align.reshape(1, 2, 128), f32)

    def pack_w(Wg):
        w = Wg.reshape(2, 128, 2, 128, 3, 3)  # [oi, o, ci, i, kh, kw]
        w = w.transpose(3, 4, 5, 2, 0, 1)  # [i, kh, kw, ci, oi, o]
        return np.ascontiguousarray(w.reshape(128, 9, 2, 2, 128), f32)

    w1 = pack_w(np.asarray(W_gen1, f32))
    w2 = pack_w(np.asarray(W_gen2, f32))
    b1 = np.ascontiguousarray(b_gen1.reshape(2, 128).T, f32)
    b2 = np.ascontiguousarray(b_gen2.reshape(2, 128).T, f32)
    eye8 = np.eye(8, dtype=f32)

    in_maps = []
    for c in range(N_CORES):
        rows = np.arange(8 * c - 2, 8 * c + 10)
        valid = (rows >= 0) & (rows < H)
        vr = rows[valid]
        xs = np.zeros((N, 128, 12, 64), f32)
        xs[:, :, valid] = preds_S[:, :, vr, :] * mat[vr][None, None]
        xsm = np.zeros((1, 12, 64), f32)
        xsm[0, valid] = mat[vr]
        rm = np.broadcast_to(
            np.array([1.0 if c > 0 else 0.0, 1.0 if c < 7 else 0.0], f32), (128, 2)
        )
        cst = np.concatenate([wa, b1, b2, rm], axis=1).astype(f32)
        slab = preds_T[:, :, 8 * c : 8 * c + RS, :].reshape(N, 2, 128, RS, 64)
        p = np.ascontiguousarray(slab.transpose(2, 0, 1, 3, 4), f32)
        in_maps.append(
            {
                "xs": xs, "xsm": xsm, "cst": cst, "wam": wam, "w1": w1, "w2": w2,
                "p": p, "eye8": eye8,
            }
        )
    return in_maps


def _make_runner(nc, n_cores):
    """Build a cached jitted SPMD runner (same mechanics as
    bass2jax.run_bass_via_pjrt, but reusable across calls)."""
    import jax
    from jax.experimental.shard_map import shard_map
    from jax.sharding import Mesh, PartitionSpec
    from concourse import bass2jax

    bass2jax.install_neuronx_cc_hook()
    assert nc.dbg_addr is None
    partition_name = nc.partition_id_tensor.name if nc.partition_id_tensor else None

    in_names, out_names, out_avals = [], [], []
    for alloc in nc.m.functions[0].allocations:
        if not isinstance(alloc, mybir.MemoryLocationSet):
            continue
        name = alloc.memorylocations[0].name
        if alloc.kind == "ExternalInput":
            if name != partition_name:
                in_names.append(name)
        elif alloc.kind == "ExternalOutput":
            out_names.append(name)
            out_avals.append(
                jax.core.ShapedArray(tuple(alloc.tensor_shape), mybir.dt.np(alloc.dtype))
            )
    n_params = len(in_names)
    n_outs = len(out_avals)
    all_names = tuple(in_names + out_names)
    if partition_name is not None:
        all_names = all_names + (partition_name,)
    donate = tuple(range(n_params, n_params + n_outs))

    def _body(*args):
        operands = list(args)
        if partition_name is not None:
            operands.append(bass2jax.partition_id_tensor())
        outs = bass2jax._bass_exec_p.bind(
            *operands,
            out_avals=tuple(out_avals),
            in_names=all_names,
            out_names=tuple(out_names),
            lowering_input_output_aliases=(),
            sim_require_finite=True,
            sim_require_nnan=True,
            nc=nc,
        )
        return tuple(outs)

    # unique per-runner name: the remote compile cache appears to key on the
    # jit module name, so distinct programs must not both be "jit__body"
    import hashlib

    _body.__name__ = "_body_" + hashlib.sha256(nc.to_json_bytes()).hexdigest()[:10]
    _body.__qualname__ = _body.__name__

    devices = jax.devices()[:n_cores]
    mesh = Mesh(np.asarray(devices), ("core",))
    in_specs = (PartitionSpec("core"),) * (n_params + n_outs)
    out_specs = (PartitionSpec("core"),) * n_outs
    sharded = jax.jit(
        shard_map(_body, mesh=mesh, in_specs=in_specs, out_specs=out_specs, check_rep=False),
        donate_argnums=donate,
        keep_unused=True,
    )

    def _collect(out_arrs):
        return [
            {
                k: np.asarray(out_arrs[i]).reshape(n_cores, *out_avals[i].shape)[c]
                for i, k in enumerate(out_names)
            }
            for c in range(n_cores)
        ]

    def run(in_maps):
        concat_in = [
            np.concatenate([np.asarray(in_maps[c][k]) for c in range(n_cores)], axis=0)
            for k in in_names
        ]
        concat_zeros = [
            np.zeros((n_cores * a.shape[0], *a.shape[1:]), a.dtype) for a in out_avals
        ]
        return _collect(sharded(*concat_in, *concat_zeros))

    def device_prep(in_maps):
        from jax.sharding import NamedSharding
        sh = NamedSharding(mesh, PartitionSpec("core"))
        return [
            jax.device_put(
                np.concatenate([np.asarray(in_maps[c][k]) for c in range(n_cores)], axis=0), sh
            )
            for k in in_names
        ]

    def device_call(dev_args, block=True):
        concat_zeros = [
            np.zeros((n_cores * a.shape[0], *a.shape[1:]), a.dtype) for a in out_avals
        ]
        out_arrs = sharded(*dev_args, *concat_zeros)
        if block:
            jax.block_until_ready(out_arrs)
        return out_arrs

    run.device_prep = device_prep
    run.device_call = device_call
    return run


def kernel(preds_S, preds_T, W_align, b_align, W_gen1, b_gen1, W_gen2, b_gen2):
    global LAST_RESULTS
    preds_S = np.asarray(preds_S, np.float32)
    preds_T = np.asarray(preds_T, np.float32)
    in_maps = _prep_inputs_v2(
        preds_S, preds_T,
        np.asarray(W_align, np.float32), np.asarray(b_align, np.float32),
        np.asarray(W_gen1, np.float32), np.asarray(b_gen1, np.float32),
        np.asarray(W_gen2, np.float32), np.asarray(b_gen2, np.float32),
    )
    for m in in_maps:
        m["vtag"] = np.zeros((1, 1), np.float32)
    if "run" not in _NC_CACHE:
        _NC_CACHE["run"] = _make_runner(_build_v2(), N_CORES)
    results = _NC_CACHE["run"](in_maps)
    LAST_RESULTS = results
    return np.float32(results[0]["loss"][0, 0])



# revision 50
# speedup vs baseline: 1.0378x; 1.0378x over previous
"""Trainium2 Bass kernel for nn_FeatureLossOursBMSE.

Model: s = conv1x1(preds_S) -> masked by checkerboard -> conv3x3 -> relu ->
conv3x3 = new_fea (t). Then pairwise Gram q[i,j] = <p_i, t_j> over D=C*H*W,
logits = -0.5*sq/64, ce = mean_i(logsumexp_j - diag), loss = ce*16*2e-5.
||p_i||^2 cancels exactly in (logsumexp_j logits[i,:] - logits[i,i]), so only
q[i,j] and ||t_j||^2 are needed:
  L[i,j] = q[i,j]/64 - ||t_j||^2/128;  ce_i = logsumexp_j L[i,j] - L[i,i].

Sharding: 8 cores, horizontal slab of 8 image rows per core, all 8 images.
Each core computes conv stack on its slab (with halo rows computed locally),
partial Gram q and ||t||^2 over its slab (D-sharded contraction), then one
72-float AllReduce and a replicated 8x8 softmax-CE tail.

Conv implementation: fp32r matmuls (full-rate on TRN2 for free>=256),
width-padded feature tiles [128, rows, 66] with zeroed border columns, taps
shift the rhs AP by kw in {0,1,2}; PSUM accumulates 18 matmuls per output
block. Checkerboard mask and align-conv bias are folded host-side: the mask
commutes with the 1x1 conv, and the bias rides an extra contraction row
whose input channel is the (row-validity-masked) mask itself.
"""

import numpy as np
from contextlib import ExitStack

import concourse.bass as bass
import concourse.mybir as mybir
import concourse.tile as tile
from concourse import bacc
from concourse.bass_utils import run_bass_kernel_spmd

F32 = mybir.dt.float32
F32R = mybir.dt.float32r
BF16 = mybir.dt.bfloat16
AF = mybir.ActivationFunctionType
ALU = mybir.AluOpType

N_CORES = 8
N, CS, CT, H, W = 8, 128, 256, 64, 64
RS = H // N_CORES  # slab rows per core = 8
NOISE_VAR = 64.0
ALPHA_MGD = 2e-05

LAST_RESULTS = None
_NC_CACHE = {}


def _build(repeats=1, use_collective=True, skip_gram=False, skip_conv=False, skip_amr=False, vtag_len=None, loop_n=None, tail_in_loop=False, tail_reps=None, psum_bufs=6, depth=2, dump=False):
    n_dev = N_CORES if use_collective else 1
    nc = bacc.Bacc("TRN2", target_bir_lowering=False, debug=False, num_devices=n_dev)
    # dummy input whose shape encodes the build variant: the remote compile
    # cache hashes HLO structure only (not backend_config), so structurally
    # identical variants would collide on the same cached NEFF
    if vtag_len is None:
        vtag_len = repeats
    vtag_in = nc.dram_tensor("vtag", [1, vtag_len], F32, kind="ExternalInput").ap()
    xs_in = nc.dram_tensor("xs", [N, 128, 12, 64], F32R, kind="ExternalInput").ap()
    xsm_in = nc.dram_tensor("xsm", [1, 12, 64], F32R, kind="ExternalInput").ap()
    cst_in = nc.dram_tensor("cst", [128, 262], F32R, kind="ExternalInput").ap()
    wam_in = nc.dram_tensor("wam", [1, 2, 128], F32R, kind="ExternalInput").ap()
    w1_in = nc.dram_tensor("w1", [128, 9, 2, 2, 128], F32R, kind="ExternalInput").ap()
    w2_in = nc.dram_tensor("w2", [128, 9, 2, 2, 128], F32R, kind="ExternalInput").ap()
    p_in = nc.dram_tensor("p", [128, N, 2, RS, 64], F32, kind="ExternalInput").ap()
    eye_in = nc.dram_tensor("eye8", [8, 8], F32, kind="ExternalInput").ap()
    loss_out = nc.dram_tensor("loss", [1, 1], F32, kind="ExternalOutput").ap()
    if dump:
        t_dbg = nc.dram_tensor("t_dbg", [N, 128, 2, RS, 64], F32, kind="ExternalOutput").ap()
        q_dbg = nc.dram_tensor("q_dbg", [1, 72], F32, kind="ExternalOutput").ap()
        mk_dbg = nc.dram_tensor("mk_dbg", [2, 128, 12, 66], F32, kind="ExternalOutput").ap()
        r1_dbg = nc.dram_tensor("r1_dbg", [2, 128, 10, 66], F32, kind="ExternalOutput").ap()

    with tile.TileContext(nc) as tc:
        with ExitStack() as ctx:
            wpool = ctx.enter_context(tc.tile_pool(name="weights", bufs=1))
            xpool = ctx.enter_context(tc.tile_pool(name="xs", bufs=3))
            fpool = ctx.enter_context(tc.tile_pool(name="feat", bufs=1))
            ppool = ctx.enter_context(tc.tile_pool(name="p", bufs=1))
            spool = ctx.enter_context(tc.tile_pool(name="scratch", bufs=2))
            cpool = ctx.enter_context(tc.tile_pool(name="ce", bufs=1))
            psum = ctx.enter_context(tc.tile_pool(name="psum", bufs=psum_bufs, space="PSUM"))
            cps = ctx.enter_context(tc.tile_pool(name="ceps", bufs=1, space="PSUM"))
            dram = ctx.enter_context(tc.tile_pool(name="dram", bufs=1, space="DRAM"))

            # ---- weights / constants (one blob DMA) ----
            cst_t = wpool.tile([128, 262], F32R)
            wam_t = wpool.tile([1, 2, 128], F32R)
            w1_t = wpool.tile([128, 9, 2, 2, 128], F32R)
            w2_t = wpool.tile([128, 9, 2, 2, 128], F32R)
            eye_t = wpool.tile([8, 8], F32)
            xsm_t = wpool.tile([1, 12, 64], F32R)
            nc.sync.dma_start(out=cst_t[:], in_=cst_in)
            wa_t = cst_t[:, 0:256].rearrange("p (a b) -> p a b", a=2)
            b1_t = cst_t[:, 256:258].bitcast(F32)
            b2_t = cst_t[:, 258:260].bitcast(F32)
            rm_t = cst_t[:, 260:262].bitcast(F32)
            nc.sync.dma_start(out=wam_t[:], in_=wam_in)
            nc.sync.dma_start(out=eye_t[:], in_=eye_in)
            nc.sync.dma_start(out=xsm_t[:], in_=xsm_in)

            p_t = ppool.tile([128, N, 2, RS, 64], F32)

            def _dma_w1():
                nc.sync.dma_start(out=w1_t[:], in_=w1_in)

            def _dma_w2p():
                nc.sync.dma_start(out=w2_t[:], in_=w2_in)
                nc.sync.dma_start(out=p_t[:], in_=p_in)

            def _emit_big_dmas():
                _dma_w1()
                _dma_w2p()

            ones = cpool.tile([128, 1], F32)
            nc.vector.memset(ones[:], 1.0)
            ones_row = cpool.tile([1, 8], F32)
            nc.vector.memset(ones_row[:], 1.0)

            # ---- persistent feature tiles (border cols pre-zeroed) ----
            masked = [
                [
                    fpool.tile([128, 12, 66], F32R, name=f"mk{par}_{oi}", tag=f"mk{par}_{oi}")
                    for oi in range(2)
                ]
                for par in range(depth)
            ]
            relu1 = [
                [
                    fpool.tile([128, 10, 66], F32R, name=f"r1{par}_{oi}", tag=f"r1{par}_{oi}")
                    for oi in range(2)
                ]
                for par in range(depth)
            ]
            t_tiles = [
                fpool.tile([128, 2, RS, 64], F32, name=f"t{n}", tag=f"t{n}")
                for n in range(N)
            ]
            zcol = cpool.tile([128, 12], F32)
            nc.vector.memset(zcol[:], 0.0)
            for par in range(depth):
                for oi in range(2):
                    nc.scalar.copy(masked[par][oi][:, :, 0:1], zcol[:, 0:12, None])
                    nc.scalar.copy(masked[par][oi][:, :, 65:66], zcol[:, 0:12, None])
                    nc.scalar.copy(relu1[par][oi][:, :, 0:1], zcol[:, 0:10, None])
                    nc.scalar.copy(relu1[par][oi][:, :, 65:66], zcol[:, 0:10, None])

            # gram accumulators: acc[:, n*8+i], tnorm[:, n]
            acc = cpool.tile([128, 64], F32)
            tnorm = cpool.tile([128, 8], F32)
            if skip_gram or skip_amr or skip_conv:
                nc.vector.memset(acc[:], 0.0)
                nc.vector.memset(tnorm[:], 1.0)
            part = cps.tile([1, 72], F32, name="part", tag="cepart")

            # ---- per-image stages ----
            def conv1x1(n):
                xs_t = xpool.tile([128, 12, 64], F32R, name="xs_t", tag="xs_t")
                nc.sync.dma_start(out=xs_t[:], in_=xs_in[n])
                mk = masked[n % depth]
                for oi in range(2):
                    for r0, R in ((0, 8), (8, 4)):
                        ps = psum.tile([128, 512], F32, name="ps_c1", tag="ps")
                        ps3 = ps[:, 0 : R * 64].rearrange("p (r c) -> p r c", r=R)
                        nc.tensor.matmul(
                            ps3, wa_t[:, oi, :], xs_t[:, r0 : r0 + R, :],
                            start=True, stop=False,
                        )
                        nc.tensor.matmul(
                            ps3, wam_t[:, oi, :], xsm_t[:, r0 : r0 + R, :],
                            start=False, stop=True,
                        )
                        nc.scalar.copy(mk[oi][:, r0 : r0 + R, 1:65], ps3)

            def conv3x3(w_t, src, b_t, n, rows_out, fb, dst_fn, func):
                # dst_fn(oi) -> (out_ap rows x 64). src: 2-chunk padded tiles.
                for oi in range(2):
                    for r0, R in fb:
                        ps = psum.tile([128, 512], F32, name="ps_g", tag="ps")
                        ps3 = ps[:, 0 : R * 64].rearrange("p (r c) -> p r c", r=R)
                        k = 0
                        for t in range(9):
                            kh, kw = t // 3, t % 3
                            for ci in range(2):
                                nc.tensor.matmul(
                                    ps3,
                                    w_t[:, t, ci, oi, :],
                                    src[ci][:, r0 + kh : r0 + kh + R, kw : kw + 64],
                                    start=(k == 0),
                                    stop=(k == 17),
                                )
                                k += 1
                        nc.scalar.activation(
                            dst_fn(oi, r0, R), ps3, func, bias=b_t[:, oi : oi + 1],
                        )

            def gen1(n):
                mk, r1 = masked[n % depth], relu1[n % depth]
                conv3x3(
                    w1_t, mk, b1_t, n, 10, ((0, 5), (5, 5)),
                    lambda oi, r0, R: r1[oi][:, r0 : r0 + R, 1:65],
                    AF.Relu,
                )
                # zero out-of-image halo rows (rows 0 and 9) via rowmask
                for oi in range(2):
                    hal = r1[oi][:, 0:10:9, :]
                    nc.vector.tensor_tensor(
                        out=hal, in0=hal,
                        in1=rm_t[:, :, None].to_broadcast([128, 2, 66]),
                        op=ALU.mult,
                    )

            def gen2(n):
                r1 = relu1[n % depth]
                conv3x3(
                    w2_t, r1, b2_t, n, RS, ((0, 4), (4, 4)),
                    lambda oi, r0, R: t_tiles[n][:, oi, r0 : r0 + R, :],
                    AF.Identity,
                )

            def gram(n, split=False):
                tt = t_tiles[n]
                sq = spool.tile([128, 1024], F32, name="sq_s", tag="sq_s")
                nc.scalar.activation(
                    sq[:].rearrange("p (a b) -> p a b", a=2), tt[:],
                    AF.Square, accum_out=tnorm[:, n : n + 1],
                )
                if skip_amr:
                    return
                if not split:
                    for i in range(N):
                        gs = spool.tile([128, 1024], F32, name="gr_s", tag="gr_s")
                        col = i * 8 + n
                        nc.vector.affine_mul_reduce(
                            out=gs[:].rearrange("p (a b) -> p a b", a=2),
                            accum_out=acc[:, col : col + 1],
                            in0=p_t[:, i].rearrange("p a r c -> p a (r c)"),
                            in1=tt[:].rearrange("p a r c -> p a (r c)"),
                            scale=1.0,
                            bias=0.0,
                        )
                    return
                # split: one AMR per (quarter, i); partial sums in acc4, then
                # a 4->1 add chain per i on the vector engine
                acc4 = cpool.tile([128, 8, 4], F32, name="acc4", tag="acc4")
                for q in range(4):
                    oi, r0 = q // 2, (q % 2) * 4
                    for i in range(N):
                        gs = spool.tile([128, 1024], F32, name="gr_s", tag="gr_s")
                        nc.vector.affine_mul_reduce(
                            out=gs[:, 0:256].rearrange("p (r c) -> p r c", r=4),
                            accum_out=acc4[:, i, q : q + 1],
                            in0=p_t[:, i, oi, r0 : r0 + 4],
                            in1=tt[:, oi, r0 : r0 + 4],
                            scale=1.0,
                            bias=0.0,
                        )
                a2 = cpool.tile([128, 8, 2], F32, name="acc2q", tag="acc2q")
                nc.vector.tensor_add(a2[:], acc4[:, :, 0:2], acc4[:, :, 2:4])
                accv = acc[:].rearrange("p (i j) -> p i j", i=8)
                nc.vector.tensor_add(accv[:, :, n], a2[:, :, 0], a2[:, :, 1])


            def _ce_tail():
                # ---- CE tail (replicated on every core) ----
                nc.tensor.matmul(part[:, 0:64], ones[:], acc[:], start=True, stop=False)
                nc.tensor.matmul(part[:, 64:72], ones[:], tnorm[:], start=False, stop=True)
                part_sb = cpool.tile([1, 72], F32)
                nc.scalar.copy(part_sb[:], part[:])
                if dump:
                    nc.sync.dma_start(out=q_dbg, in_=part_sb[:])
                    for n in range(N):
                        nc.sync.dma_start(out=t_dbg[n], in_=t_tiles[n][:])
                    for oi in range(2):
                        nc.sync.dma_start(out=mk_dbg[oi], in_=masked[0][oi][:].bitcast(F32))
                        nc.sync.dma_start(out=r1_dbg[oi], in_=relu1[0][oi][:].bitcast(F32))

                cc_in = dram.tile([1, 72], F32)
                cc_out = dram.tile([1, 72], F32)
                nc.sync.dma_start(out=cc_in[:], in_=part_sb[:])
                if use_collective:
                    nc.gpsimd.collective_compute(
                        "AllReduce", ALU.add, replica_groups=[list(range(N_CORES))],
                        ins=[cc_in[:].opt()], outs=[cc_out[:].opt()],
                    )
                else:
                    nc.gpsimd.dma_start(out=cc_out[:], in_=cc_in[:])

                q_sb = cpool.tile([8, 8], F32)
                tn_row = cpool.tile([1, 8], F32)
                nc.sync.dma_start(
                    out=q_sb[:], in_=cc_out[:, 0:64].rearrange("a (i j) -> (a i) j", i=8)
                )
                nc.sync.dma_start(out=tn_row[:], in_=cc_out[:, 64:72])

                # L[i,j] = q/64 - tn[j]/128 via PE broadcast
                q_s = cpool.tile([8, 8], F32)
                nc.scalar.mul(q_s[:], q_sb[:], 1.0 / NOISE_VAR)
                tn_neg = cpool.tile([1, 8], F32)
                nc.scalar.mul(tn_neg[:], tn_row[:], -1.0 / (2.0 * NOISE_VAR))
                L_ps = cps.tile([8, 8], F32, name="L_ps", tag="ceps")
                nc.tensor.matmul(L_ps[:], eye_t[:], q_s[:], start=True, stop=False)
                nc.tensor.matmul(L_ps[:], ones_row[:], tn_neg[:], start=False, stop=True)
                L = cpool.tile([8, 8], F32)
                nc.vector.tensor_copy(L[:], L_ps[:])

                m = cpool.tile([8, 1], F32)
                nc.vector.reduce_max(m[:], L[:], axis=mybir.AxisListType.X)
                negm = cpool.tile([8, 1], F32)
                nc.scalar.mul(negm[:], m[:], -1.0)
                e = cpool.tile([8, 8], F32)
                nc.scalar.activation(e[:], L[:], AF.Exp, bias=negm[:, 0:1], scale=1.0)
                s = cpool.tile([8, 1], F32)
                nc.vector.reduce_sum(s[:], e[:], axis=mybir.AxisListType.X)
                ln_s = cpool.tile([8, 1], F32)
                nc.scalar.activation(ln_s[:], s[:], AF.Ln)

                # diag(L) via eye mask (avoids strided DRAM readbacks)
                ldm = cpool.tile([8, 8], F32)
                nc.vector.tensor_mul(ldm[:], L[:], eye_t[:])
                ld = cpool.tile([8, 1], F32)
                nc.vector.reduce_sum(ld[:], ldm[:], axis=mybir.AxisListType.X)
                ce = cpool.tile([8, 1], F32)
                nc.vector.tensor_add(ce[:], m[:], ln_s[:])
                nc.vector.tensor_sub(ce[:], ce[:], ld[:])

                lp = cps.tile([1, 1], F32, name="lp", tag="ceps")
                nc.tensor.matmul(lp[:], ce[:], ones[0:8, :], start=True, stop=True)
                l_sb = cpool.tile([1, 1], F32)
                # loss = sum_i ce_i / N * (2*NOISE_VAR/N) * ALPHA
                nc.scalar.mul(l_sb[:], lp[:], (2.0 * NOISE_VAR / (N * N)) * ALPHA_MGD)
                nc.sync.dma_start(out=loss_out, in_=l_sb[:])


            if skip_conv:
                for n in range(N):
                    nc.vector.memset(t_tiles[n][:], 0.125)

            def _body_once(stage_dmas=False):
                if skip_conv:
                    if stage_dmas:
                        _emit_big_dmas()
                    for n in range(N):
                        gram(n)
                else:
                    conv1x1(0)
                    if stage_dmas:
                        _dma_w1()
                    for k in range(1, depth):
                        conv1x1(k)
                    for n in range(N):
                        gen1(n)
                        if stage_dmas and n == 0:
                            _dma_w2p()
                        gen2(n)
                        if n + depth < N:
                            conv1x1(n + depth)
                        if not skip_gram:
                            gram(n, split=(n == N - 1))

            if tail_reps is not None:
                nc.vector.memset(acc[:], 0.0)
                nc.vector.memset(tnorm[:], 1.0)
                for _ in range(tail_reps):
                    _ce_tail()
            elif loop_n is not None:
                _emit_big_dmas()
                if tail_in_loop:
                    with tc.For_i(0, loop_n, 1) as _i:
                        _body_once()
                        _ce_tail()
                else:
                    with tc.For_i(0, loop_n, 1) as _i:
                        _body_once()
                    _ce_tail()
            else:
                for _rep in range(repeats):
                    _body_once(stage_dmas=(_rep == 0))
                    _ce_tail()

    nc.compile()
    return nc


def _build_v2(repeats=1, use_collective=True, vtag_len=None, loop_n=None,
              reps_per_iter=1, tail_in_loop=False, tail_reps=None, psum_bufs=6,
              dump=False, skip_gram=False, skip_conv=False, skip_amr=False,
              bf16_gram=False):
    """Parity-optimized build.

    The checkerboard mask zeroes even sites of s, so (a) conv1x1 computes
    only odd-site outputs (half the rows), stored column-deinterleaved as
    M[p, img, row, slot] with slot j <-> padded col cc = 2j + (row%2); and
    (b) gen1 skips taps whose input site is even: for an output block of
    fixed (row-parity rho, col-parity gam), only taps with
    (kh+kw)%2 == (rho+gam)%2 contribute (4 or 5 of 9). Halves gen1 PE work
    and instruction count. Blocks span an image PAIR to keep the fp32r
    full-rate free>=256 threshold. gen2 is dense and unchanged
    arithmetically, but also paired (free 512).
    """
    n_dev = N_CORES if use_collective else 1
    nc = bacc.Bacc("TRN2", target_bir_lowering=False, debug=False, num_devices=n_dev)
    if vtag_len is None:
        vtag_len = repeats
    vtag_in = nc.dram_tensor("vtag", [1, vtag_len], F32, kind="ExternalInput").ap()
    xs_in = nc.dram_tensor("xs", [N, 128, 12, 64], F32R, kind="ExternalInput").ap()
    cst_in = nc.dram_tensor("cst", [128, 268], F32R, kind="ExternalInput").ap()
    w1_in = nc.dram_tensor("w1", [128, 9, 2, 2, 128], F32R, kind="ExternalInput").ap()
    w2_in = nc.dram_tensor("w2", [128, 9, 2, 2, 128], F32R, kind="ExternalInput").ap()
    GDT = BF16 if bf16_gram else F32
    p_in = nc.dram_tensor("p", [128, N, 2, RS, 64], GDT, kind="ExternalInput").ap()
    eye_in = nc.dram_tensor("eye8", [8, 8], F32, kind="ExternalInput").ap()
    loss_out = nc.dram_tensor("loss", [1, 1], F32, kind="ExternalOutput").ap()
    if dump:
        m_dbg = nc.dram_tensor("m_dbg", [2, 128, 2, 12, 33], F32, kind="ExternalOutput").ap()
        r1_dbg = nc.dram_tensor("r1_dbg", [2, 128, 2, 10, 66], F32, kind="ExternalOutput").ap()
        t_dbg = nc.dram_tensor("t_dbg", [N, 128, 2, RS, 64], F32, kind="ExternalOutput").ap()
        q_dbg = nc.dram_tensor("q_dbg", [1, 72], F32, kind="ExternalOutput").ap()

    TAPS = {par: [(kh, kw) for kh in range(3) for kw in range(3)
                  if (kh + kw) % 2 == par] for par in (0, 1)}

    with tile.TileContext(nc) as tc:
        with ExitStack() as ctx:
            wpool = ctx.enter_context(tc.tile_pool(name="weights", bufs=1))
            xpool = ctx.enter_context(tc.tile_pool(name="xs", bufs=4))
            fpool = ctx.enter_context(tc.tile_pool(name="feat", bufs=1))
            ppool = ctx.enter_context(tc.tile_pool(name="p", bufs=1))
            spool = ctx.enter_context(tc.tile_pool(name="scratch", bufs=2))
            qpool = ctx.enter_context(tc.tile_pool(name="qscratch", bufs=2))
            cpool = ctx.enter_context(tc.tile_pool(name="ce", bufs=1))
            # PSUM banks: 4 (conv1x1+gen1, 512 f32) + 1x2 (gen2, 1024 f32)
            # + 1 (ce tail) = 7 of 8
            psum = ctx.enter_context(tc.tile_pool(name="psum", bufs=4, space="PSUM"))
            gpsum = ctx.enter_context(tc.tile_pool(name="gpsum", bufs=1, space="PSUM"))
            cps = ctx.enter_context(tc.tile_pool(name="ceps", bufs=1, space="PSUM"))
            dram = ctx.enter_context(tc.tile_pool(name="dram", bufs=1, space="DRAM"))

            # ---- weights / constants ----
            cst_t = wpool.tile([128, 268], F32R)
            w1_t = wpool.tile([128, 9, 2, 2, 128], F32R)
            w2_t = wpool.tile([128, 9, 2, 2, 128], F32R)
            eye_t = wpool.tile([8, 8], F32)
            nc.sync.dma_start(out=cst_t[:], in_=cst_in)
            wa_t = cst_t[:, 0:256].rearrange("p (a b) -> p a b", a=2)
            b1_t = cst_t[:, 256:258].bitcast(F32)
            b2_t = cst_t[:, 258:260].bitcast(F32)
            rm_t = cst_t[:, 260:262].bitcast(F32)
            ba_t = cst_t[:, 262:264].bitcast(F32)
            hm_t = cst_t[:, 264:268].bitcast(F32)
            nc.sync.dma_start(out=eye_t[:], in_=eye_in)

            p_t = ppool.tile([128, N, 2, RS, 64], GDT)

            def _emit_big_dmas():
                nc.sync.dma_start(out=w1_t[:], in_=w1_in)
                nc.sync.dma_start(out=w2_t[:], in_=w2_in)
                nc.sync.dma_start(out=p_t[:], in_=p_in)

            ones = cpool.tile([128, 1], F32)
            nc.vector.memset(ones[:], 1.0)
            ones_row = cpool.tile([1, 8], F32)
            nc.vector.memset(ones_row[:], 1.0)

            # ---- persistent feature tiles ----
            Ms = [[fpool.tile([128, 2, 12, 33], F32R, name=f"M{d}_{oi}", tag=f"M{d}_{oi}")
                   for oi in range(2)] for d in range(2)]
            R1s = [[fpool.tile([128, 2, 10, 66], F32R, name=f"R{d}_{oi}", tag=f"R{d}_{oi}")
                    for oi in range(2)] for d in range(2)]
            Ts = [fpool.tile([128, 2, 2, RS, 64], GDT, name=f"T{g}", tag=f"T{g}")
                  for g in range(4)]
            zf = cpool.tile([128, 1320], F32)
            nc.vector.memset(zf[:], 0.0)
            for d in range(2):
                for oi in range(2):
                    nc.scalar.copy(
                        Ms[d][oi][:].rearrange("p u r j -> p (u r j)"), zf[:, 0:792])
                    nc.scalar.copy(
                        R1s[d][oi][:].rearrange("p u r j -> p (u r j)"), zf[:, 0:1320])

            acc = cpool.tile([128, 64], F32)
            tnorm = cpool.tile([128, 8], F32)
            part = cps.tile([1, 72], F32, name="part", tag="ceps")

            # ---- stages ----
            def load_xs(g):
                XS = xpool.tile([128, 2, 12, 64], F32R, name="xs_t", tag="xs_t")
                nc.sync.dma_start(out=XS[:, 0], in_=xs_in[2 * g])
                nc.sync.dma_start(out=XS[:, 1], in_=xs_in[2 * g + 1])
                return XS

            def conv1x1(g, XS):
                d = g % 2
                for oi in range(2):
                    for rho in range(2):
                        ps = psum.tile([128, 512], F32, name="ps_c1", tag="ps")
                        psv = ps[:, 0:384].rearrange("p (u r c) -> p u r c", u=2, r=6)
                        nc.tensor.matmul(
                            psv, wa_t[:, oi, :],
                            XS[:, :, rho:12:2, (1 - rho):64:2],
                            start=True, stop=True,
                        )
                        j0 = 1 - rho
                        # per-partition b_align bias rides the ACT drain; the
                        # (core 0/7 only) out-of-image halo rows get bias too
                        # and are re-zeroed below on Pool
                        nc.scalar.activation(
                            Ms[d][oi][:, :, rho:12:2, j0:j0 + 32], psv,
                            AF.Identity, bias=ba_t[:, oi:oi + 1],
                        )
                    for r0, h0 in ((0, 0), (10, 2)):
                        hal = Ms[d][oi][:, :, r0:r0 + 2, :]
                        nc.gpsimd.tensor_tensor(
                            out=hal, in0=hal,
                            in1=hm_t[:, None, h0:h0 + 2, None]
                                .to_broadcast([128, 2, 2, 33]),
                            op=ALU.mult,
                        )

            def gen1(g):
                d = g % 2
                for oi in range(2):
                    for rho in range(2):
                        for gam in range(2):
                            taps = TAPS[(rho + gam) % 2]
                            ps = psum.tile([128, 512], F32, name="ps_g1", tag="ps")
                            psv = ps[:, 0:320].rearrange("p (u r c) -> p u r c", u=2, r=5)
                            k, last = 0, len(taps) * 2 - 1
                            for kh, kw in taps:
                                j0 = (gam + kw - ((rho + kh) % 2)) // 2
                                for ci in range(2):
                                    nc.tensor.matmul(
                                        psv, w1_t[:, kh * 3 + kw, ci, oi, :],
                                        Ms[d][ci][:, :, rho + kh:rho + kh + 9:2, j0:j0 + 32],
                                        start=(k == 0), stop=(k == last),
                                    )
                                    k += 1
                            nc.scalar.activation(
                                R1s[d][oi][:, :, rho:10:2, gam + 1:gam + 65:2],
                                psv, AF.Relu, bias=b1_t[:, oi:oi + 1],
                            )
                    hal = R1s[d][oi][:, :, 0:10:9, :]
                    nc.gpsimd.tensor_tensor(
                        out=hal, in0=hal,
                        in1=rm_t[:, None, :, None].to_broadcast([128, 2, 2, 66]),
                        op=ALU.mult,
                    )

            def gen2(g):
                d = g % 2
                for oi in range(2):
                    # one [128,1024] psum = 2 banks; block b fills bank b
                    ps = gpsum.tile([128, 1024], F32, name="ps_g2", tag="gps")
                    ps5 = ps[:].rearrange("p (b u r c) -> p b u r c", b=2, u=2, r=4)
                    for b in range(2):
                        k = 0
                        for t9 in range(9):
                            kh, kw = t9 // 3, t9 % 3
                            for ci in range(2):
                                nc.tensor.matmul(
                                    ps5[:, b], w2_t[:, t9, ci, oi, :],
                                    R1s[d][ci][:, :, b * 4 + kh:b * 4 + kh + 4, kw:kw + 64],
                                    start=(k == 0), stop=(k == 17),
                                )
                                k += 1
                    for u in range(2):
                        nc.scalar.activation(
                            Ts[g][:, u, oi], ps5[:, :, u], AF.Identity,
                            bias=b2_t[:, oi:oi + 1],
                        )

            def gram(n, split=False):
                tt = Ts[n // 2][:, n % 2]
                sq = spool.tile([128, 1024], F32, name="sq_s", tag="sq_s")
                nc.scalar.activation(
                    sq[:].rearrange("p (a b) -> p a b", a=2), tt,
                    AF.Square, accum_out=tnorm[:, n:n + 1],
                )
                if skip_amr:
                    return
                if not split:
                    for i in range(N):
                        gs = spool.tile([128, 1024], GDT, name="gr_s", tag="gr_s")
                        col = i * 8 + n
                        nc.vector.affine_mul_reduce(
                            out=gs[:].rearrange("p (a b) -> p a b", a=2),
                            accum_out=acc[:, col:col + 1],
                            in0=p_t[:, i].rearrange("p a r c -> p a (r c)"),
                            in1=tt.rearrange("p a r c -> p a (r c)"),
                            scale=1.0, bias=0.0,
                        )
                    return
                # split per oi half so the oi=0 dots start before gen2's
                # oi=1 matmuls finish (shortens the end-of-body DVE drain)
                a2 = cpool.tile([128, 8, 2], F32, name="acc2h", tag="acc2h")
                for oi in range(2):
                    for i in range(N):
                        gs = spool.tile([128, 1024], GDT, name="gr_s", tag="gr_s")
                        nc.vector.affine_mul_reduce(
                            out=gs[:, 0:512].rearrange("p (r c) -> p r c", r=RS),
                            accum_out=a2[:, i, oi:oi + 1],
                            in0=p_t[:, i, oi],
                            in1=tt[:, oi],
                            scale=1.0, bias=0.0,
                        )
                accv = acc[:].rearrange("p (i j) -> p i j", i=8)
                nc.vector.tensor_add(accv[:, :, n], a2[:, :, 0], a2[:, :, 1])

            def _ce_tail():
                nc.tensor.matmul(part[:, 0:64], ones[:], acc[:], start=True, stop=False)
                nc.tensor.matmul(part[:, 64:72], ones[:], tnorm[:], start=False, stop=True)
                part_sb = cpool.tile([1, 72], F32)
                nc.scalar.copy(part_sb[:], part[:])
                if dump:
                    nc.sync.dma_start(out=q_dbg, in_=part_sb[:])
                    for n in range(N):
                        nc.sync.dma_start(out=t_dbg[n], in_=Ts[n // 2][:, n % 2])
                    for oi in range(2):
                        nc.sync.dma_start(out=m_dbg[oi], in_=Ms[0][oi][:].bitcast(F32))
                        nc.sync.dma_start(out=r1_dbg[oi], in_=R1s[0][oi][:].bitcast(F32))

                cc_in = dram.tile([1, 72], F32)
                cc_out = dram.tile([1, 72], F32)
                nc.sync.dma_start(out=cc_in[:], in_=part_sb[:])
                if use_collective:
                    nc.gpsimd.collective_compute(
                        "AllReduce", ALU.add, replica_groups=[list(range(N_CORES))],
                        ins=[cc_in[:].opt()], outs=[cc_out[:].opt()],
                    )
                else:
                    nc.gpsimd.dma_start(out=cc_out[:], in_=cc_in[:])

                q_sb = cpool.tile([8, 8], F32)
                tn_row = cpool.tile([1, 8], F32)
                nc.sync.dma_start(
                    out=q_sb[:], in_=cc_out[:, 0:64].rearrange("a (i j) -> (a i) j", i=8)
                )
                nc.sync.dma_start(out=tn_row[:], in_=cc_out[:, 64:72])

                q_s = cpool.tile([8, 8], F32)
                nc.scalar.mul(q_s[:], q_sb[:], 1.0 / NOISE_VAR)
                tn_neg = cpool.tile([1, 8], F32)
                nc.scalar.mul(tn_neg[:], tn_row[:], -1.0 / (2.0 * NOISE_VAR))
                L_ps = cps.tile([8, 8], F32, name="L_ps", tag="ceps")
                nc.tensor.matmul(L_ps[:], eye_t[:], q_s[:], start=True, stop=False)
                nc.tensor.matmul(L_ps[:], ones_row[:], tn_neg[:], start=False, stop=True)
                L = cpool.tile([8, 8], F32)
                nc.vector.tensor_copy(L[:], L_ps[:])

                m = cpool.tile([8, 1], F32)
                nc.vector.reduce_max(m[:], L[:], axis=mybir.AxisListType.X)
                negm = cpool.tile([8, 1], F32)
                nc.scalar.mul(negm[:], m[:], -1.0)
                e = cpool.tile([8, 8], F32)
                nc.scalar.activation(e[:], L[:], AF.Exp, bias=negm[:, 0:1], scale=1.0)
                s = cpool.tile([8, 1], F32)
                nc.vector.reduce_sum(s[:], e[:], axis=mybir.AxisListType.X)
                ln_s = cpool.tile([8, 1], F32)
                nc.scalar.activation(ln_s[:], s[:], AF.Ln)

                ldm = cpool.tile([8, 8], F32)
                nc.vector.tensor_mul(ldm[:], L[:], eye_t[:])
                ld = cpool.tile([8, 1], F32)
                nc.vector.reduce_sum(ld[:], ldm[:], axis=mybir.AxisListType.X)
                ce = cpool.tile([8, 1], F32)
                nc.vector.tensor_add(ce[:], m[:], ln_s[:])
                nc.vector.tensor_sub(ce[:], ce[:], ld[:])

                lp = cps.tile([1, 1], F32, name="lp", tag="ceps")
                nc.tensor.matmul(lp[:], ce[:], ones[0:8, :], start=True, stop=True)
                l_sb = cpool.tile([1, 1], F32)
                nc.scalar.mul(l_sb[:], lp[:], (2.0 * NOISE_VAR / (N * N)) * ALPHA_MGD)
                nc.sync.dma_start(out=loss_out, in_=l_sb[:])

            if skip_gram or skip_conv or skip_amr:
                nc.vector.memset(acc[:], 0.0)
                nc.vector.memset(tnorm[:], 1.0)
            if skip_conv:
                for g in range(4):
                    nc.vector.memset(Ts[g][:], 0.125)

            def _body_once():
                if skip_conv:
                    for n in range(N):
                        gram(n)
                    return
                xs_tiles = [load_xs(g) for g in range(4)]
                conv1x1(0, xs_tiles[0])
                for g in range(4):
                    gen1(g)
                    if g + 1 < 4:
                        conv1x1(g + 1, xs_tiles[g + 1])
                    gen2(g)
                    if not skip_gram:
                        gram(2 * g, split=(g == 3))
                        gram(2 * g + 1, split=(g == 3))

            _emit_big_dmas()
            if tail_reps is not None:
                nc.vector.memset(acc[:], 0.0)
                nc.vector.memset(tnorm[:], 1.0)
                for _ in range(tail_reps):
                    _ce_tail()
            elif loop_n is not None:
                if tail_in_loop:
                    with tc.For_i(0, loop_n, 1) as _i:
                        _body_once()
                        _ce_tail()
                else:
                    with tc.For_i(0, loop_n, 1) as _i:
                        for _r in range(reps_per_iter):
                            _body_once()
                    _ce_tail()
            else:
                for _rep in range(repeats):
                    _body_once()
                    _ce_tail()

    nc.compile()
    return nc


def _prep_inputs_v2(preds_S, preds_T, W_align, b_align, W_gen1, b_gen1, W_gen2,
                    b_gen2, bf16_gram=False):
    f32 = np.float32
    pdt = mybir.dt.np(BF16) if bf16_gram else f32

    wa = np.ascontiguousarray(W_align[:, :, 0, 0].T.reshape(128, 256), f32)
    ba = np.ascontiguousarray(np.asarray(b_align, f32).reshape(2, 128).T, f32)

    def pack_w(Wg):
        w = Wg.reshape(2, 128, 2, 128, 3, 3)
        w = w.transpose(3, 4, 5, 2, 0, 1)
        return np.ascontiguousarray(w.reshape(128, 9, 2, 2, 128), f32)

    w1 = pack_w(np.asarray(W_gen1, f32))
    w2 = pack_w(np.asarray(W_gen2, f32))
    b1 = np.ascontiguousarray(b_gen1.reshape(2, 128).T, f32)
    b2 = np.ascontiguousarray(b_gen2.reshape(2, 128).T, f32)
    eye8 = np.eye(8, dtype=f32)

    in_maps = []
    for c in range(N_CORES):
        rows = np.arange(8 * c - 2, 8 * c + 10)
        valid = (rows >= 0) & (rows < H)
        vr = rows[valid]
        xs = np.zeros((N, 128, 12, 64), f32)
        xs[:, :, valid] = preds_S[:, :, vr, :]
        rm = np.broadcast_to(
            np.array([1.0 if c > 0 else 0.0, 1.0 if c < 7 else 0.0], f32), (128, 2)
        )
        # halo-row validity for M rows 0,1,10,11 (conv1x1 bias fixup)
        hm = np.broadcast_to(
            valid[[0, 1, 10, 11]].astype(f32), (128, 4)
        )
        cst = np.concatenate([wa, b1, b2, rm, ba, hm], axis=1).astype(f32)
        slab = preds_T[:, :, 8 * c: 8 * c + RS, :].reshape(N, 2, 128, RS, 64)
        p = np.ascontiguousarray(slab.transpose(2, 0, 1, 3, 4)).astype(pdt)
        in_maps.append(
            {
                "xs": xs, "cst": cst, "w1": w1, "w2": w2,
                "p": p, "eye8": eye8,
            }
        )
    return in_maps


def _prep_inputs(preds_S, preds_T, W_align, b_align, W_gen1, b_gen1, W_gen2, b_gen2):
    f32 = np.float32
    mat = ((np.arange(H)[:, None] + np.arange(W)[None, :]) % 2).astype(f32)

    wa = np.ascontiguousarray(W_align[:, :, 0, 0].T.reshape(128, 256), f32)
    wam = np.ascontiguousarray(b_align.reshape(1, 2, 128), f32)

    def pack_w(Wg):
        w = Wg.reshape(2, 128, 2, 128, 3, 3)  # [oi, o, ci, i, kh, kw]
        w = w.transpose(3, 4, 5, 2, 0, 1)  # [i, kh, kw, ci, oi, o]
        return np.ascontiguousarray(w.reshape(128, 9, 2, 2, 128), f32)

    w1 = pack_w(np.asarray(W_gen1, f32))
    w2 = pack_w(np.asarray(W_gen2, f32))
    b1 = np.ascontiguousarray(b_gen1.reshape(2, 128).T, f32)
    b2 = np.ascontiguousarray(b_gen2.reshape(2, 128).T, f32)
    eye8 = np.eye(8, dtype=f32)

    in_maps = []
    for c in range(N_CORES):
        rows = np.arange(8 * c - 2, 8 * c + 10)
        valid = (rows >= 0) & (rows < H)
        vr = rows[valid]
        xs = np.zeros((N, 128, 12, 64), f32)
        xs[:, :, valid] = preds_S[:, :, vr, :] * mat[vr][None, None]
        xsm = np.zeros((1, 12, 64), f32)
        xsm[0, valid] = mat[vr]
        rm = np.broadcast_to(
            np.array([1.0 if c > 0 else 0.0, 1.0 if c < 7 else 0.0], f32), (128, 2)
        )
        cst = np.concatenate([wa, b1, b2, rm], axis=1).astype(f32)
        slab = preds_T[:, :, 8 * c : 8 * c + RS, :].reshape(N, 2, 128, RS, 64)
        p = np.ascontiguousarray(slab.transpose(2, 0, 1, 3, 4), f32)
        in_maps.append(
            {
                "xs": xs, "xsm": xsm, "cst": cst, "wam": wam, "w1": w1, "w2": w2,
                "p": p, "eye8": eye8,
            }
        )
    return in_maps


def _make_runner(nc, n_cores):
    """Build a cached jitted SPMD runner (same mechanics as
    bass2jax.run_bass_via_pjrt, but reusable across calls)."""
    import jax
    from jax.experimental.shard_map import shard_map
    from jax.sharding import Mesh, PartitionSpec
    from concourse import bass2jax

    bass2jax.install_neuronx_cc_hook()
    assert nc.dbg_addr is None
    partition_name = nc.partition_id_tensor.name if nc.partition_id_tensor else None

    in_names, out_names, out_avals = [], [], []
    for alloc in nc.m.functions[0].allocations:
        if not isinstance(alloc, mybir.MemoryLocationSet):
            continue
        name = alloc.memorylocations[0].name
        if alloc.kind == "ExternalInput":
            if name != partition_name:
                in_names.append(name)
        elif alloc.kind == "ExternalOutput":
            out_names.append(name)
            out_avals.append(
                jax.core.ShapedArray(tuple(alloc.tensor_shape), mybir.dt.np(alloc.dtype))
            )
    n_params = len(in_names)
    n_outs = len(out_avals)
    all_names = tuple(in_names + out_names)
    if partition_name is not None:
        all_names = all_names + (partition_name,)
    donate = tuple(range(n_params, n_params + n_outs))

    def _body(*args):
        operands = list(args)
        if partition_name is not None:
            operands.append(bass2jax.partition_id_tensor())
        outs = bass2jax._bass_exec_p.bind(
            *operands,
            out_avals=tuple(out_avals),
            in_names=all_names,
            out_names=tuple(out_names),
            lowering_input_output_aliases=(),
            sim_require_finite=True,
            sim_require_nnan=True,
            nc=nc,
        )
        return tuple(outs)

    # unique per-runner name: the remote compile cache appears to key on the
    # jit module name, so distinct programs must not both be "jit__body"
    import hashlib

    _body.__name__ = "_body_" + hashlib.sha256(nc.to_json_bytes()).hexdigest()[:10]
    _body.__qualname__ = _body.__name__

    devices = jax.devices()[:n_cores]
    mesh = Mesh(np.asarray(devices), ("core",))
    in_specs = (PartitionSpec("core"),) * (n_params + n_outs)
    out_specs = (PartitionSpec("core"),) * n_outs
    sharded = jax.jit(
        shard_map(_body, mesh=mesh, in_specs=in_specs, out_specs=out_specs, check_rep=False),
        donate_argnums=donate,
        keep_unused=True,
    )

    def _collect(out_arrs):
        return [
            {
                k: np.asarray(out_arrs[i]).reshape(n_cores, *out_avals[i].shape)[c]
                for i, k in enumerate(out_names)
            }
            for c in range(n_cores)
        ]

    def run(in_maps):
        concat_in = [
            np.concatenate([np.asarray(in_maps[c][k]) for c in range(n_cores)], axis=0)
            for k in in_names
        ]
        concat_zeros = [
            np.zeros((n_cores * a.shape[0], *a.shape[1:]), a.dtype) for a in out_avals
        ]
        return _collect(sharded(*concat_in, *concat_zeros))

    def device_prep(in_maps):
        from jax.sharding import NamedSharding
        sh = NamedSharding(mesh, PartitionSpec("core"))
        return [
            jax.device_put(
                np.concatenate([np.asarray(in_maps[c][k]) for c in range(n_cores)], axis=0), sh
            )
            for k in in_names
        ]

    def device_call(dev_args, block=True):
        concat_zeros = [
            np.zeros((n_cores * a.shape[0], *a.shape[1:]), a.dtype) for a in out_avals
        ]
        out_arrs = sharded(*dev_args, *concat_zeros)
        if block:
            jax.block_until_ready(out_arrs)
        return out_arrs

    run.device_prep = device_prep
    run.device_call = device_call
    return run


def kernel(preds_S, preds_T, W_align, b_align, W_gen1, b_gen1, W_gen2, b_gen2):
    global LAST_RESULTS
    preds_S = np.asarray(preds_S, np.float32)
    preds_T = np.asarray(preds_T, np.float32)
    in_maps = _prep_inputs_v2(
        preds_S, preds_T,
        np.asarray(W_align, np.float32), np.asarray(b_align, np.float32),
        np.asarray(W_gen1, np.float32), np.asarray(b_gen1, np.float32),
        np.asarray(W_gen2, np.float32), np.asarray(b_gen2, np.float32),
    )
    for m in in_maps:
        m["vtag"] = np.zeros((1, 1), np.float32)
    if "run" not in _NC_CACHE:
        _NC_CACHE["run"] = _make_runner(_build_v2(), N_CORES)
    results = _NC_CACHE["run"](in_maps)
    LAST_RESULTS = results
    return np.float32(results[0]["loss"][0, 0])



# revision 51
# speedup vs baseline: 1.0398x; 1.0020x over previous
"""Trainium2 Bass kernel for nn_FeatureLossOursBMSE.

Model: s = conv1x1(preds_S) -> masked by checkerboard -> conv3x3 -> relu ->
conv3x3 = new_fea (t). Then pairwise Gram q[i,j] = <p_i, t_j> over D=C*H*W,
logits = -0.5*sq/64, ce = mean_i(logsumexp_j - diag), loss = ce*16*2e-5.
||p_i||^2 cancels exactly in (logsumexp_j logits[i,:] - logits[i,i]), so only
q[i,j] and ||t_j||^2 are needed:
  L[i,j] = q[i,j]/64 - ||t_j||^2/128;  ce_i = logsumexp_j L[i,j] - L[i,i].

Sharding: 8 cores, horizontal slab of 8 image rows per core, all 8 images.
Each core computes conv stack on its slab (with halo rows computed locally),
partial Gram q and ||t||^2 over its slab (D-sharded contraction), then one
72-float AllReduce and a replicated 8x8 softmax-CE tail.

Conv implementation: fp32r matmuls (full-rate on TRN2 for free>=256),
width-padded feature tiles [128, rows, 66] with zeroed border columns, taps
shift the rhs AP by kw in {0,1,2}; PSUM accumulates 18 matmuls per output
block. Checkerboard mask and align-conv bias are folded host-side: the mask
commutes with the 1x1 conv, and the bias rides an extra contraction row
whose input channel is the (row-validity-masked) mask itself.
"""

import numpy as np
from contextlib import ExitStack

import concourse.bass as bass
import concourse.mybir as mybir
import concourse.tile as tile
from concourse import bacc
from concourse.bass_utils import run_bass_kernel_spmd

F32 = mybir.dt.float32
F32R = mybir.dt.float32r
BF16 = mybir.dt.bfloat16
AF = mybir.ActivationFunctionType
ALU = mybir.AluOpType

N_CORES = 8
N, CS, CT, H, W = 8, 128, 256, 64, 64
RS = H // N_CORES  # slab rows per core = 8
NOISE_VAR = 64.0
ALPHA_MGD = 2e-05

LAST_RESULTS = None
_NC_CACHE = {}


def _build(repeats=1, use_collective=True, skip_gram=False, skip_conv=False, skip_amr=False, vtag_len=None, loop_n=None, tail_in_loop=False, tail_reps=None, psum_bufs=6, depth=2, dump=False):
    n_dev = N_CORES if use_collective else 1
    nc = bacc.Bacc("TRN2", target_bir_lowering=False, debug=False, num_devices=n_dev)
    # dummy input whose shape encodes the build variant: the remote compile
    # cache hashes HLO structure only (not backend_config), so structurally
    # identical variants would collide on the same cached NEFF
    if vtag_len is None:
        vtag_len = repeats
    vtag_in = nc.dram_tensor("vtag", [1, vtag_len], F32, kind="ExternalInput").ap()
    xs_in = nc.dram_tensor("xs", [N, 128, 12, 64], F32R, kind="ExternalInput").ap()
    xsm_in = nc.dram_tensor("xsm", [1, 12, 64], F32R, kind="ExternalInput").ap()
    cst_in = nc.dram_tensor("cst", [128, 262], F32R, kind="ExternalInput").ap()
    wam_in = nc.dram_tensor("wam", [1, 2, 128], F32R, kind="ExternalInput").ap()
    w1_in = nc.dram_tensor("w1", [128, 9, 2, 2, 128], F32R, kind="ExternalInput").ap()
    w2_in = nc.dram_tensor("w2", [128, 9, 2, 2, 128], F32R, kind="ExternalInput").ap()
    p_in = nc.dram_tensor("p", [128, N, 2, RS, 64], F32, kind="ExternalInput").ap()
    eye_in = nc.dram_tensor("eye8", [8, 8], F32, kind="ExternalInput").ap()
    loss_out = nc.dram_tensor("loss", [1, 1], F32, kind="ExternalOutput").ap()
    if dump:
        t_dbg = nc.dram_tensor("t_dbg", [N, 128, 2, RS, 64], F32, kind="ExternalOutput").ap()
        q_dbg = nc.dram_tensor("q_dbg", [1, 72], F32, kind="ExternalOutput").ap()
        mk_dbg = nc.dram_tensor("mk_dbg", [2, 128, 12, 66], F32, kind="ExternalOutput").ap()
        r1_dbg = nc.dram_tensor("r1_dbg", [2, 128, 10, 66], F32, kind="ExternalOutput").ap()

    with tile.TileContext(nc) as tc:
        with ExitStack() as ctx:
            wpool = ctx.enter_context(tc.tile_pool(name="weights", bufs=1))
            xpool = ctx.enter_context(tc.tile_pool(name="xs", bufs=3))
            fpool = ctx.enter_context(tc.tile_pool(name="feat", bufs=1))
            ppool = ctx.enter_context(tc.tile_pool(name="p", bufs=1))
            spool = ctx.enter_context(tc.tile_pool(name="scratch", bufs=2))
            cpool = ctx.enter_context(tc.tile_pool(name="ce", bufs=1))
            psum = ctx.enter_context(tc.tile_pool(name="psum", bufs=psum_bufs, space="PSUM"))
            cps = ctx.enter_context(tc.tile_pool(name="ceps", bufs=1, space="PSUM"))
            dram = ctx.enter_context(tc.tile_pool(name="dram", bufs=1, space="DRAM"))

            # ---- weights / constants (one blob DMA) ----
            cst_t = wpool.tile([128, 262], F32R)
            wam_t = wpool.tile([1, 2, 128], F32R)
            w1_t = wpool.tile([128, 9, 2, 2, 128], F32R)
            w2_t = wpool.tile([128, 9, 2, 2, 128], F32R)
            eye_t = wpool.tile([8, 8], F32)
            xsm_t = wpool.tile([1, 12, 64], F32R)
            nc.sync.dma_start(out=cst_t[:], in_=cst_in)
            wa_t = cst_t[:, 0:256].rearrange("p (a b) -> p a b", a=2)
            b1_t = cst_t[:, 256:258].bitcast(F32)
            b2_t = cst_t[:, 258:260].bitcast(F32)
            rm_t = cst_t[:, 260:262].bitcast(F32)
            nc.sync.dma_start(out=wam_t[:], in_=wam_in)
            nc.sync.dma_start(out=eye_t[:], in_=eye_in)
            nc.sync.dma_start(out=xsm_t[:], in_=xsm_in)

            p_t = ppool.tile([128, N, 2, RS, 64], F32)

            def _dma_w1():
                nc.sync.dma_start(out=w1_t[:], in_=w1_in)

            def _dma_w2p():
                nc.sync.dma_start(out=w2_t[:], in_=w2_in)
                nc.sync.dma_start(out=p_t[:], in_=p_in)

            def _emit_big_dmas():
                _dma_w1()
                _dma_w2p()

            ones = cpool.tile([128, 1], F32)
            nc.vector.memset(ones[:], 1.0)
            ones_row = cpool.tile([1, 8], F32)
            nc.vector.memset(ones_row[:], 1.0)

            # ---- persistent feature tiles (border cols pre-zeroed) ----
            masked = [
                [
                    fpool.tile([128, 12, 66], F32R, name=f"mk{par}_{oi}", tag=f"mk{par}_{oi}")
                    for oi in range(2)
                ]
                for par in range(depth)
            ]
            relu1 = [
                [
                    fpool.tile([128, 10, 66], F32R, name=f"r1{par}_{oi}", tag=f"r1{par}_{oi}")
                    for oi in range(2)
                ]
                for par in range(depth)
            ]
            t_tiles = [
                fpool.tile([128, 2, RS, 64], F32, name=f"t{n}", tag=f"t{n}")
                for n in range(N)
            ]
            zcol = cpool.tile([128, 12], F32)
            nc.vector.memset(zcol[:], 0.0)
            for par in range(depth):
                for oi in range(2):
                    nc.scalar.copy(masked[par][oi][:, :, 0:1], zcol[:, 0:12, None])
                    nc.scalar.copy(masked[par][oi][:, :, 65:66], zcol[:, 0:12, None])
                    nc.scalar.copy(relu1[par][oi][:, :, 0:1], zcol[:, 0:10, None])
                    nc.scalar.copy(relu1[par][oi][:, :, 65:66], zcol[:, 0:10, None])

            # gram accumulators: acc[:, n*8+i], tnorm[:, n]
            acc = cpool.tile([128, 64], F32)
            tnorm = cpool.tile([128, 8], F32)
            if skip_gram or skip_amr or skip_conv:
                nc.vector.memset(acc[:], 0.0)
                nc.vector.memset(tnorm[:], 1.0)
            part = cps.tile([1, 72], F32, name="part", tag="cepart")

            # ---- per-image stages ----
            def conv1x1(n):
                xs_t = xpool.tile([128, 12, 64], F32R, name="xs_t", tag="xs_t")
                nc.sync.dma_start(out=xs_t[:], in_=xs_in[n])
                mk = masked[n % depth]
                for oi in range(2):
                    for r0, R in ((0, 8), (8, 4)):
                        ps = psum.tile([128, 512], F32, name="ps_c1", tag="ps")
                        ps3 = ps[:, 0 : R * 64].rearrange("p (r c) -> p r c", r=R)
                        nc.tensor.matmul(
                            ps3, wa_t[:, oi, :], xs_t[:, r0 : r0 + R, :],
                            start=True, stop=False,
                        )
                        nc.tensor.matmul(
                            ps3, wam_t[:, oi, :], xsm_t[:, r0 : r0 + R, :],
                            start=False, stop=True,
                        )
                        nc.scalar.copy(mk[oi][:, r0 : r0 + R, 1:65], ps3)

            def conv3x3(w_t, src, b_t, n, rows_out, fb, dst_fn, func):
                # dst_fn(oi) -> (out_ap rows x 64). src: 2-chunk padded tiles.
                for oi in range(2):
                    for r0, R in fb:
                        ps = psum.tile([128, 512], F32, name="ps_g", tag="ps")
                        ps3 = ps[:, 0 : R * 64].rearrange("p (r c) -> p r c", r=R)
                        k = 0
                        for t in range(9):
                            kh, kw = t // 3, t % 3
                            for ci in range(2):
                                nc.tensor.matmul(
                                    ps3,
                                    w_t[:, t, ci, oi, :],
                                    src[ci][:, r0 + kh : r0 + kh + R, kw : kw + 64],
                                    start=(k == 0),
                                    stop=(k == 17),
                                )
                                k += 1
                        nc.scalar.activation(
                            dst_fn(oi, r0, R), ps3, func, bias=b_t[:, oi : oi + 1],
                        )

            def gen1(n):
                mk, r1 = masked[n % depth], relu1[n % depth]
                conv3x3(
                    w1_t, mk, b1_t, n, 10, ((0, 5), (5, 5)),
                    lambda oi, r0, R: r1[oi][:, r0 : r0 + R, 1:65],
                    AF.Relu,
                )
                # zero out-of-image halo rows (rows 0 and 9) via rowmask
                for oi in range(2):
                    hal = r1[oi][:, 0:10:9, :]
                    nc.vector.tensor_tensor(
                        out=hal, in0=hal,
                        in1=rm_t[:, :, None].to_broadcast([128, 2, 66]),
                        op=ALU.mult,
                    )

            def gen2(n):
                r1 = relu1[n % depth]
                conv3x3(
                    w2_t, r1, b2_t, n, RS, ((0, 4), (4, 4)),
                    lambda oi, r0, R: t_tiles[n][:, oi, r0 : r0 + R, :],
                    AF.Identity,
                )

            def gram(n, split=False):
                tt = t_tiles[n]
                sq = spool.tile([128, 1024], F32, name="sq_s", tag="sq_s")
                nc.scalar.activation(
                    sq[:].rearrange("p (a b) -> p a b", a=2), tt[:],
                    AF.Square, accum_out=tnorm[:, n : n + 1],
                )
                if skip_amr:
                    return
                if not split:
                    for i in range(N):
                        gs = spool.tile([128, 1024], F32, name="gr_s", tag="gr_s")
                        col = i * 8 + n
                        nc.vector.affine_mul_reduce(
                            out=gs[:].rearrange("p (a b) -> p a b", a=2),
                            accum_out=acc[:, col : col + 1],
                            in0=p_t[:, i].rearrange("p a r c -> p a (r c)"),
                            in1=tt[:].rearrange("p a r c -> p a (r c)"),
                            scale=1.0,
                            bias=0.0,
                        )
                    return
                # split: one AMR per (quarter, i); partial sums in acc4, then
                # a 4->1 add chain per i on the vector engine
                acc4 = cpool.tile([128, 8, 4], F32, name="acc4", tag="acc4")
                for q in range(4):
                    oi, r0 = q // 2, (q % 2) * 4
                    for i in range(N):
                        gs = spool.tile([128, 1024], F32, name="gr_s", tag="gr_s")
                        nc.vector.affine_mul_reduce(
                            out=gs[:, 0:256].rearrange("p (r c) -> p r c", r=4),
                            accum_out=acc4[:, i, q : q + 1],
                            in0=p_t[:, i, oi, r0 : r0 + 4],
                            in1=tt[:, oi, r0 : r0 + 4],
                            scale=1.0,
                            bias=0.0,
                        )
                a2 = cpool.tile([128, 8, 2], F32, name="acc2q", tag="acc2q")
                nc.vector.tensor_add(a2[:], acc4[:, :, 0:2], acc4[:, :, 2:4])
                accv = acc[:].rearrange("p (i j) -> p i j", i=8)
                nc.vector.tensor_add(accv[:, :, n], a2[:, :, 0], a2[:, :, 1])


            def _ce_tail():
                # ---- CE tail (replicated on every core) ----
                nc.tensor.matmul(part[:, 0:64], ones[:], acc[:], start=True, stop=False)
                nc.tensor.matmul(part[:, 64:72], ones[:], tnorm[:], start=False, stop=True)
                part_sb = cpool.tile([1, 72], F32)
                nc.scalar.copy(part_sb[:], part[:])
                if dump:
                    nc.sync.dma_start(out=q_dbg, in_=part_sb[:])
                    for n in range(N):
                        nc.sync.dma_start(out=t_dbg[n], in_=t_tiles[n][:])
                    for oi in range(2):
                        nc.sync.dma_start(out=mk_dbg[oi], in_=masked[0][oi][:].bitcast(F32))
                        nc.sync.dma_start(out=r1_dbg[oi], in_=relu1[0][oi][:].bitcast(F32))

                cc_in = dram.tile([1, 72], F32)
                cc_out = dram.tile([1, 72], F32)
                nc.sync.dma_start(out=cc_in[:], in_=part_sb[:])
                if use_collective:
                    nc.gpsimd.collective_compute(
                        "AllReduce", ALU.add, replica_groups=[list(range(N_CORES))],
                        ins=[cc_in[:].opt()], outs=[cc_out[:].opt()],
                    )
                else:
                    nc.gpsimd.dma_start(out=cc_out[:], in_=cc_in[:])

                q_sb = cpool.tile([8, 8], F32)
                tn_row = cpool.tile([1, 8], F32)
                nc.sync.dma_start(
                    out=q_sb[:], in_=cc_out[:, 0:64].rearrange("a (i j) -> (a i) j", i=8)
                )
                nc.sync.dma_start(out=tn_row[:], in_=cc_out[:, 64:72])

                # L[i,j] = q/64 - tn[j]/128 via PE broadcast
                q_s = cpool.tile([8, 8], F32)
                nc.scalar.mul(q_s[:], q_sb[:], 1.0 / NOISE_VAR)
                tn_neg = cpool.tile([1, 8], F32)
                nc.scalar.mul(tn_neg[:], tn_row[:], -1.0 / (2.0 * NOISE_VAR))
                L_ps = cps.tile([8, 8], F32, name="L_ps", tag="ceps")
                nc.tensor.matmul(L_ps[:], eye_t[:], q_s[:], start=True, stop=False)
                nc.tensor.matmul(L_ps[:], ones_row[:], tn_neg[:], start=False, stop=True)
                L = cpool.tile([8, 8], F32)
                nc.vector.tensor_copy(L[:], L_ps[:])

                m = cpool.tile([8, 1], F32)
                nc.vector.reduce_max(m[:], L[:], axis=mybir.AxisListType.X)
                negm = cpool.tile([8, 1], F32)
                nc.scalar.mul(negm[:], m[:], -1.0)
                e = cpool.tile([8, 8], F32)
                nc.scalar.activation(e[:], L[:], AF.Exp, bias=negm[:, 0:1], scale=1.0)
                s = cpool.tile([8, 1], F32)
                nc.vector.reduce_sum(s[:], e[:], axis=mybir.AxisListType.X)
                ln_s = cpool.tile([8, 1], F32)
                nc.scalar.activation(ln_s[:], s[:], AF.Ln)

                # diag(L) via eye mask (avoids strided DRAM readbacks)
                ldm = cpool.tile([8, 8], F32)
                nc.vector.tensor_mul(ldm[:], L[:], eye_t[:])
                ld = cpool.tile([8, 1], F32)
                nc.vector.reduce_sum(ld[:], ldm[:], axis=mybir.AxisListType.X)
                ce = cpool.tile([8, 1], F32)
                nc.vector.tensor_add(ce[:], m[:], ln_s[:])
                nc.vector.tensor_sub(ce[:], ce[:], ld[:])

                lp = cps.tile([1, 1], F32, name="lp", tag="ceps")
                nc.tensor.matmul(lp[:], ce[:], ones[0:8, :], start=True, stop=True)
                l_sb = cpool.tile([1, 1], F32)
                # loss = sum_i ce_i / N * (2*NOISE_VAR/N) * ALPHA
                nc.scalar.mul(l_sb[:], lp[:], (2.0 * NOISE_VAR / (N * N)) * ALPHA_MGD)
                nc.sync.dma_start(out=loss_out, in_=l_sb[:])


            if skip_conv:
                for n in range(N):
                    nc.vector.memset(t_tiles[n][:], 0.125)

            def _body_once(stage_dmas=False):
                if skip_conv:
                    if stage_dmas:
                        _emit_big_dmas()
                    for n in range(N):
                        gram(n)
                else:
                    conv1x1(0)
                    if stage_dmas:
                        _dma_w1()
                    for k in range(1, depth):
                        conv1x1(k)
                    for n in range(N):
                        gen1(n)
                        if stage_dmas and n == 0:
                            _dma_w2p()
                        gen2(n)
                        if n + depth < N:
                            conv1x1(n + depth)
                        if not skip_gram:
                            gram(n, split=(n == N - 1))

            if tail_reps is not None:
                nc.vector.memset(acc[:], 0.0)
                nc.vector.memset(tnorm[:], 1.0)
                for _ in range(tail_reps):
                    _ce_tail()
            elif loop_n is not None:
                _emit_big_dmas()
                if tail_in_loop:
                    with tc.For_i(0, loop_n, 1) as _i:
                        _body_once()
                        _ce_tail()
                else:
                    with tc.For_i(0, loop_n, 1) as _i:
                        _body_once()
                    _ce_tail()
            else:
                for _rep in range(repeats):
                    _body_once(stage_dmas=(_rep == 0))
                    _ce_tail()

    nc.compile()
    return nc


def _build_v2(repeats=1, use_collective=True, vtag_len=None, loop_n=None,
              reps_per_iter=1, tail_in_loop=False, tail_reps=None, psum_bufs=6,
              dump=False, skip_gram=False, skip_conv=False, skip_amr=False,
              bf16_gram=False):
    """Parity-optimized build.

    The checkerboard mask zeroes even sites of s, so (a) conv1x1 computes
    only odd-site outputs (half the rows), stored column-deinterleaved as
    M[p, img, row, slot] with slot j <-> padded col cc = 2j + (row%2); and
    (b) gen1 skips taps whose input site is even: for an output block of
    fixed (row-parity rho, col-parity gam), only taps with
    (kh+kw)%2 == (rho+gam)%2 contribute (4 or 5 of 9). Halves gen1 PE work
    and instruction count. Blocks span an image PAIR to keep the fp32r
    full-rate free>=256 threshold. gen2 is dense and unchanged
    arithmetically, but also paired (free 512).
    """
    n_dev = N_CORES if use_collective else 1
    nc = bacc.Bacc("TRN2", target_bir_lowering=False, debug=False, num_devices=n_dev)
    if vtag_len is None:
        vtag_len = repeats
    vtag_in = nc.dram_tensor("vtag", [1, vtag_len], F32, kind="ExternalInput").ap()
    xs_in = nc.dram_tensor("xs", [N, 128, 12, 64], F32R, kind="ExternalInput").ap()
    cst_in = nc.dram_tensor("cst", [128, 268], F32R, kind="ExternalInput").ap()
    w1_in = nc.dram_tensor("w1", [128, 9, 2, 2, 128], F32R, kind="ExternalInput").ap()
    w2_in = nc.dram_tensor("w2", [128, 9, 2, 2, 128], F32R, kind="ExternalInput").ap()
    GDT = BF16 if bf16_gram else F32
    p_in = nc.dram_tensor("p", [128, N, 2, RS, 64], GDT, kind="ExternalInput").ap()
    eye_in = nc.dram_tensor("eye8", [8, 8], F32, kind="ExternalInput").ap()
    loss_out = nc.dram_tensor("loss", [1, 1], F32, kind="ExternalOutput").ap()
    if dump:
        m_dbg = nc.dram_tensor("m_dbg", [2, 128, 2, 12, 33], F32, kind="ExternalOutput").ap()
        r1_dbg = nc.dram_tensor("r1_dbg", [2, 128, 2, 10, 66], F32, kind="ExternalOutput").ap()
        t_dbg = nc.dram_tensor("t_dbg", [N, 128, 2, RS, 64], F32, kind="ExternalOutput").ap()
        q_dbg = nc.dram_tensor("q_dbg", [1, 72], F32, kind="ExternalOutput").ap()

    TAPS = {par: [(kh, kw) for kh in range(3) for kw in range(3)
                  if (kh + kw) % 2 == par] for par in (0, 1)}

    with tile.TileContext(nc) as tc:
        with ExitStack() as ctx:
            wpool = ctx.enter_context(tc.tile_pool(name="weights", bufs=1))
            xpool = ctx.enter_context(tc.tile_pool(name="xs", bufs=4))
            fpool = ctx.enter_context(tc.tile_pool(name="feat", bufs=1))
            ppool = ctx.enter_context(tc.tile_pool(name="p", bufs=1))
            spool = ctx.enter_context(tc.tile_pool(name="scratch", bufs=2))
            qpool = ctx.enter_context(tc.tile_pool(name="qscratch", bufs=2))
            cpool = ctx.enter_context(tc.tile_pool(name="ce", bufs=1))
            # PSUM banks: 4 (conv1x1+gen1, 512 f32) + 1x2 (gen2, 1024 f32)
            # + 1 (ce tail) = 7 of 8
            psum = ctx.enter_context(tc.tile_pool(name="psum", bufs=4, space="PSUM"))
            gpsum = ctx.enter_context(tc.tile_pool(name="gpsum", bufs=1, space="PSUM"))
            cps = ctx.enter_context(tc.tile_pool(name="ceps", bufs=1, space="PSUM"))
            dram = ctx.enter_context(tc.tile_pool(name="dram", bufs=1, space="DRAM"))

            # ---- weights / constants ----
            cst_t = wpool.tile([128, 268], F32R)
            w1_t = wpool.tile([128, 9, 2, 2, 128], F32R)
            w2_t = wpool.tile([128, 9, 2, 2, 128], F32R)
            eye_t = wpool.tile([8, 8], F32)
            nc.sync.dma_start(out=cst_t[:], in_=cst_in)
            wa_t = cst_t[:, 0:256].rearrange("p (a b) -> p a b", a=2)
            b1_t = cst_t[:, 256:258].bitcast(F32)
            b2_t = cst_t[:, 258:260].bitcast(F32)
            rm_t = cst_t[:, 260:262].bitcast(F32)
            ba_t = cst_t[:, 262:264].bitcast(F32)
            hm_t = cst_t[:, 264:268].bitcast(F32)
            nc.sync.dma_start(out=eye_t[:], in_=eye_in)

            p_t = ppool.tile([128, N, 2, RS, 64], GDT)

            def _emit_big_dmas():
                nc.sync.dma_start(out=w1_t[:], in_=w1_in)
                nc.sync.dma_start(out=w2_t[:], in_=w2_in)
                nc.sync.dma_start(out=p_t[:], in_=p_in)

            ones = cpool.tile([128, 1], F32)
            nc.vector.memset(ones[:], 1.0)
            ones_row = cpool.tile([1, 8], F32)
            nc.vector.memset(ones_row[:], 1.0)

            # ---- persistent feature tiles ----
            Ms = [[fpool.tile([128, 2, 12, 33], F32R, name=f"M{d}_{oi}", tag=f"M{d}_{oi}")
                   for oi in range(2)] for d in range(2)]
            R1s = [[fpool.tile([128, 2, 10, 66], F32R, name=f"R{d}_{oi}", tag=f"R{d}_{oi}")
                    for oi in range(2)] for d in range(2)]
            Ts = [fpool.tile([128, 2, 2, RS, 64], GDT, name=f"T{g}", tag=f"T{g}")
                  for g in range(4)]
            zf = cpool.tile([128, 1320], F32)
            nc.vector.memset(zf[:], 0.0)
            for d in range(2):
                for oi in range(2):
                    nc.scalar.copy(
                        Ms[d][oi][:].rearrange("p u r j -> p (u r j)"), zf[:, 0:792])
                    nc.scalar.copy(
                        R1s[d][oi][:].rearrange("p u r j -> p (u r j)"), zf[:, 0:1320])

            acc = cpool.tile([128, 64], F32)
            tnorm = cpool.tile([128, 8], F32)
            part = cps.tile([1, 72], F32, name="part", tag="ceps")

            # ---- stages ----
            def load_xs(g):
                XS = xpool.tile([128, 2, 12, 64], F32R, name="xs_t", tag="xs_t")
                nc.sync.dma_start(out=XS[:, 0], in_=xs_in[2 * g])
                # second image on the Pool engine's DMA queue: pair-0's two
                # loads run in parallel, halving the post-Drain stall at the
                # top of each loop iteration
                nc.gpsimd.dma_start(out=XS[:, 1], in_=xs_in[2 * g + 1])
                return XS

            def conv1x1(g, XS):
                d = g % 2
                for oi in range(2):
                    for rho in range(2):
                        ps = psum.tile([128, 512], F32, name="ps_c1", tag="ps")
                        psv = ps[:, 0:384].rearrange("p (u r c) -> p u r c", u=2, r=6)
                        nc.tensor.matmul(
                            psv, wa_t[:, oi, :],
                            XS[:, :, rho:12:2, (1 - rho):64:2],
                            start=True, stop=True,
                        )
                        j0 = 1 - rho
                        # per-partition b_align bias rides the ACT drain; the
                        # (core 0/7 only) out-of-image halo rows get bias too
                        # and are re-zeroed below on Pool
                        nc.scalar.activation(
                            Ms[d][oi][:, :, rho:12:2, j0:j0 + 32], psv,
                            AF.Identity, bias=ba_t[:, oi:oi + 1],
                        )
                    for r0, h0 in ((0, 0), (10, 2)):
                        hal = Ms[d][oi][:, :, r0:r0 + 2, :]
                        nc.gpsimd.tensor_tensor(
                            out=hal, in0=hal,
                            in1=hm_t[:, None, h0:h0 + 2, None]
                                .to_broadcast([128, 2, 2, 33]),
                            op=ALU.mult,
                        )

            def gen1(g):
                d = g % 2
                for oi in range(2):
                    for rho in range(2):
                        for gam in range(2):
                            taps = TAPS[(rho + gam) % 2]
                            ps = psum.tile([128, 512], F32, name="ps_g1", tag="ps")
                            psv = ps[:, 0:320].rearrange("p (u r c) -> p u r c", u=2, r=5)
                            k, last = 0, len(taps) * 2 - 1
                            for kh, kw in taps:
                                j0 = (gam + kw - ((rho + kh) % 2)) // 2
                                for ci in range(2):
                                    nc.tensor.matmul(
                                        psv, w1_t[:, kh * 3 + kw, ci, oi, :],
                                        Ms[d][ci][:, :, rho + kh:rho + kh + 9:2, j0:j0 + 32],
                                        start=(k == 0), stop=(k == last),
                                    )
                                    k += 1
                            nc.scalar.activation(
                                R1s[d][oi][:, :, rho:10:2, gam + 1:gam + 65:2],
                                psv, AF.Relu, bias=b1_t[:, oi:oi + 1],
                            )
                    hal = R1s[d][oi][:, :, 0:10:9, :]
                    nc.gpsimd.tensor_tensor(
                        out=hal, in0=hal,
                        in1=rm_t[:, None, :, None].to_broadcast([128, 2, 2, 66]),
                        op=ALU.mult,
                    )

            def gen2(g):
                d = g % 2
                for oi in range(2):
                    # one [128,1024] psum = 2 banks; block b fills bank b
                    ps = gpsum.tile([128, 1024], F32, name="ps_g2", tag="gps")
                    ps5 = ps[:].rearrange("p (b u r c) -> p b u r c", b=2, u=2, r=4)
                    for b in range(2):
                        k = 0
                        for t9 in range(9):
                            kh, kw = t9 // 3, t9 % 3
                            for ci in range(2):
                                nc.tensor.matmul(
                                    ps5[:, b], w2_t[:, t9, ci, oi, :],
                                    R1s[d][ci][:, :, b * 4 + kh:b * 4 + kh + 4, kw:kw + 64],
                                    start=(k == 0), stop=(k == 17),
                                )
                                k += 1
                    for u in range(2):
                        nc.scalar.activation(
                            Ts[g][:, u, oi], ps5[:, :, u], AF.Identity,
                            bias=b2_t[:, oi:oi + 1],
                        )

            def gram(n, split=False):
                tt = Ts[n // 2][:, n % 2]
                sq = spool.tile([128, 1024], F32, name="sq_s", tag="sq_s")
                nc.scalar.activation(
                    sq[:].rearrange("p (a b) -> p a b", a=2), tt,
                    AF.Square, accum_out=tnorm[:, n:n + 1],
                )
                if skip_amr:
                    return
                if not split:
                    for i in range(N):
                        gs = spool.tile([128, 1024], GDT, name="gr_s", tag="gr_s")
                        col = i * 8 + n
                        nc.vector.affine_mul_reduce(
                            out=gs[:].rearrange("p (a b) -> p a b", a=2),
                            accum_out=acc[:, col:col + 1],
                            in0=p_t[:, i].rearrange("p a r c -> p a (r c)"),
                            in1=tt.rearrange("p a r c -> p a (r c)"),
                            scale=1.0, bias=0.0,
                        )
                    return
                # split per oi half so the oi=0 dots start before gen2's
                # oi=1 matmuls finish (shortens the end-of-body DVE drain)
                a2 = cpool.tile([128, 8, 2], F32, name="acc2h", tag="acc2h")
                for oi in range(2):
                    for i in range(N):
                        gs = spool.tile([128, 1024], GDT, name="gr_s", tag="gr_s")
                        nc.vector.affine_mul_reduce(
                            out=gs[:, 0:512].rearrange("p (r c) -> p r c", r=RS),
                            accum_out=a2[:, i, oi:oi + 1],
                            in0=p_t[:, i, oi],
                            in1=tt[:, oi],
                            scale=1.0, bias=0.0,
                        )
                accv = acc[:].rearrange("p (i j) -> p i j", i=8)
                nc.vector.tensor_add(accv[:, :, n], a2[:, :, 0], a2[:, :, 1])

            def _ce_tail():
                nc.tensor.matmul(part[:, 0:64], ones[:], acc[:], start=True, stop=False)
                nc.tensor.matmul(part[:, 64:72], ones[:], tnorm[:], start=False, stop=True)
                part_sb = cpool.tile([1, 72], F32)
                nc.scalar.copy(part_sb[:], part[:])
                if dump:
                    nc.sync.dma_start(out=q_dbg, in_=part_sb[:])
                    for n in range(N):
                        nc.sync.dma_start(out=t_dbg[n], in_=Ts[n // 2][:, n % 2])
                    for oi in range(2):
                        nc.sync.dma_start(out=m_dbg[oi], in_=Ms[0][oi][:].bitcast(F32))
                        nc.sync.dma_start(out=r1_dbg[oi], in_=R1s[0][oi][:].bitcast(F32))

                cc_in = dram.tile([1, 72], F32)
                cc_out = dram.tile([1, 72], F32)
                nc.sync.dma_start(out=cc_in[:], in_=part_sb[:])
                if use_collective:
                    nc.gpsimd.collective_compute(
                        "AllReduce", ALU.add, replica_groups=[list(range(N_CORES))],
                        ins=[cc_in[:].opt()], outs=[cc_out[:].opt()],
                    )
                else:
                    nc.gpsimd.dma_start(out=cc_out[:], in_=cc_in[:])

                q_sb = cpool.tile([8, 8], F32)
                tn_row = cpool.tile([1, 8], F32)
                nc.sync.dma_start(
                    out=q_sb[:], in_=cc_out[:, 0:64].rearrange("a (i j) -> (a i) j", i=8)
                )
                nc.sync.dma_start(out=tn_row[:], in_=cc_out[:, 64:72])

                q_s = cpool.tile([8, 8], F32)
                nc.scalar.mul(q_s[:], q_sb[:], 1.0 / NOISE_VAR)
                tn_neg = cpool.tile([1, 8], F32)
                nc.scalar.mul(tn_neg[:], tn_row[:], -1.0 / (2.0 * NOISE_VAR))
                L_ps = cps.tile([8, 8], F32, name="L_ps", tag="ceps")
                nc.tensor.matmul(L_ps[:], eye_t[:], q_s[:], start=True, stop=False)
                nc.tensor.matmul(L_ps[:], ones_row[:], tn_neg[:], start=False, stop=True)
                L = cpool.tile([8, 8], F32)
                nc.vector.tensor_copy(L[:], L_ps[:])

                m = cpool.tile([8, 1], F32)
                nc.vector.reduce_max(m[:], L[:], axis=mybir.AxisListType.X)
                negm = cpool.tile([8, 1], F32)
                nc.scalar.mul(negm[:], m[:], -1.0)
                e = cpool.tile([8, 8], F32)
                nc.scalar.activation(e[:], L[:], AF.Exp, bias=negm[:, 0:1], scale=1.0)
                s = cpool.tile([8, 1], F32)
                nc.vector.reduce_sum(s[:], e[:], axis=mybir.AxisListType.X)
                ln_s = cpool.tile([8, 1], F32)
                nc.scalar.activation(ln_s[:], s[:], AF.Ln)

                ldm = cpool.tile([8, 8], F32)
                nc.vector.tensor_mul(ldm[:], L[:], eye_t[:])
                ld = cpool.tile([8, 1], F32)
                nc.vector.reduce_sum(ld[:], ldm[:], axis=mybir.AxisListType.X)
                ce = cpool.tile([8, 1], F32)
                nc.vector.tensor_add(ce[:], m[:], ln_s[:])
                nc.vector.tensor_sub(ce[:], ce[:], ld[:])

                lp = cps.tile([1, 1], F32, name="lp", tag="ceps")
                nc.tensor.matmul(lp[:], ce[:], ones[0:8, :], start=True, stop=True)
                l_sb = cpool.tile([1, 1], F32)
                nc.scalar.mul(l_sb[:], lp[:], (2.0 * NOISE_VAR / (N * N)) * ALPHA_MGD)
                nc.sync.dma_start(out=loss_out, in_=l_sb[:])

            if skip_gram or skip_conv or skip_amr:
                nc.vector.memset(acc[:], 0.0)
                nc.vector.memset(tnorm[:], 1.0)
            if skip_conv:
                for g in range(4):
                    nc.vector.memset(Ts[g][:], 0.125)

            def _body_once():
                if skip_conv:
                    for n in range(N):
                        gram(n)
                    return
                xs_tiles = [load_xs(g) for g in range(4)]
                conv1x1(0, xs_tiles[0])
                for g in range(4):
                    gen1(g)
                    if g + 1 < 4:
                        conv1x1(g + 1, xs_tiles[g + 1])
                    gen2(g)
                    if not skip_gram:
                        gram(2 * g, split=(g == 3))
                        gram(2 * g + 1, split=(g == 3))

            _emit_big_dmas()
            if tail_reps is not None:
                nc.vector.memset(acc[:], 0.0)
                nc.vector.memset(tnorm[:], 1.0)
                for _ in range(tail_reps):
                    _ce_tail()
            elif loop_n is not None:
                if tail_in_loop:
                    with tc.For_i(0, loop_n, 1) as _i:
                        _body_once()
                        _ce_tail()
                else:
                    with tc.For_i(0, loop_n, 1) as _i:
                        for _r in range(reps_per_iter):
                            _body_once()
                    _ce_tail()
            else:
                for _rep in range(repeats):
                    _body_once()
                    _ce_tail()

    nc.compile()
    return nc


def _prep_inputs_v2(preds_S, preds_T, W_align, b_align, W_gen1, b_gen1, W_gen2,
                    b_gen2, bf16_gram=False):
    f32 = np.float32
    pdt = mybir.dt.np(BF16) if bf16_gram else f32

    wa = np.ascontiguousarray(W_align[:, :, 0, 0].T.reshape(128, 256), f32)
    ba = np.ascontiguousarray(np.asarray(b_align, f32).reshape(2, 128).T, f32)

    def pack_w(Wg):
        w = Wg.reshape(2, 128, 2, 128, 3, 3)
        w = w.transpose(3, 4, 5, 2, 0, 1)
        return np.ascontiguousarray(w.reshape(128, 9, 2, 2, 128), f32)

    w1 = pack_w(np.asarray(W_gen1, f32))
    w2 = pack_w(np.asarray(W_gen2, f32))
    b1 = np.ascontiguousarray(b_gen1.reshape(2, 128).T, f32)
    b2 = np.ascontiguousarray(b_gen2.reshape(2, 128).T, f32)
    eye8 = np.eye(8, dtype=f32)

    in_maps = []
    for c in range(N_CORES):
        rows = np.arange(8 * c - 2, 8 * c + 10)
        valid = (rows >= 0) & (rows < H)
        vr = rows[valid]
        xs = np.zeros((N, 128, 12, 64), f32)
        xs[:, :, valid] = preds_S[:, :, vr, :]
        rm = np.broadcast_to(
            np.array([1.0 if c > 0 else 0.0, 1.0 if c < 7 else 0.0], f32), (128, 2)
        )
        # halo-row validity for M rows 0,1,10,11 (conv1x1 bias fixup)
        hm = np.broadcast_to(
            valid[[0, 1, 10, 11]].astype(f32), (128, 4)
        )
        cst = np.concatenate([wa, b1, b2, rm, ba, hm], axis=1).astype(f32)
        slab = preds_T[:, :, 8 * c: 8 * c + RS, :].reshape(N, 2, 128, RS, 64)
        p = np.ascontiguousarray(slab.transpose(2, 0, 1, 3, 4)).astype(pdt)
        in_maps.append(
            {
                "xs": xs, "cst": cst, "w1": w1, "w2": w2,
                "p": p, "eye8": eye8,
            }
        )
    return in_maps


def _prep_inputs(preds_S, preds_T, W_align, b_align, W_gen1, b_gen1, W_gen2, b_gen2):
    f32 = np.float32
    mat = ((np.arange(H)[:, None] + np.arange(W)[None, :]) % 2).astype(f32)

    wa = np.ascontiguousarray(W_align[:, :, 0, 0].T.reshape(128, 256), f32)
    wam = np.ascontiguousarray(b_align.reshape(1, 2, 128), f32)

    def pack_w(Wg):
        w = Wg.reshape(2, 128, 2, 128, 3, 3)  # [oi, o, ci, i, kh, kw]
        w = w.transpose(3, 4, 5, 2, 0, 1)  # [i, kh, kw, ci, oi, o]
        return np.ascontiguousarray(w.reshape(128, 9, 2, 2, 128), f32)

    w1 = pack_w(np.asarray(W_gen1, f32))
    w2 = pack_w(np.asarray(W_gen2, f32))
    b1 = np.ascontiguousarray(b_gen1.reshape(2, 128).T, f32)
    b2 = np.ascontiguousarray(b_gen2.reshape(2, 128).T, f32)
    eye8 = np.eye(8, dtype=f32)

    in_maps = []
    for c in range(N_CORES):
        rows = np.arange(8 * c - 2, 8 * c + 10)
        valid = (rows >= 0) & (rows < H)
        vr = rows[valid]
        xs = np.zeros((N, 128, 12, 64), f32)
        xs[:, :, valid] = preds_S[:, :, vr, :] * mat[vr][None, None]
        xsm = np.zeros((1, 12, 64), f32)
        xsm[0, valid] = mat[vr]
        rm = np.broadcast_to(
            np.array([1.0 if c > 0 else 0.0, 1.0 if c < 7 else 0.0], f32), (128, 2)
        )
        cst = np.concatenate([wa, b1, b2, rm], axis=1).astype(f32)
        slab = preds_T[:, :, 8 * c : 8 * c + RS, :].reshape(N, 2, 128, RS, 64)
        p = np.ascontiguousarray(slab.transpose(2, 0, 1, 3, 4), f32)
        in_maps.append(
            {
                "xs": xs, "xsm": xsm, "cst": cst, "wam": wam, "w1": w1, "w2": w2,
                "p": p, "eye8": eye8,
            }
        )
    return in_maps


def _make_runner(nc, n_cores):
    """Build a cached jitted SPMD runner (same mechanics as
    bass2jax.run_bass_via_pjrt, but reusable across calls)."""
    import jax
    from jax.experimental.shard_map import shard_map
    from jax.sharding import Mesh, PartitionSpec
    from concourse import bass2jax

    bass2jax.install_neuronx_cc_hook()
    assert nc.dbg_addr is None
    partition_name = nc.partition_id_tensor.name if nc.partition_id_tensor else None

    in_names, out_names, out_avals = [], [], []
    for alloc in nc.m.functions[0].allocations:
        if not isinstance(alloc, mybir.MemoryLocationSet):
            continue
        name = alloc.memorylocations[0].name
        if alloc.kind == "ExternalInput":
            if name != partition_name:
                in_names.append(name)
        elif alloc.kind == "ExternalOutput":
            out_names.append(name)
            out_avals.append(
                jax.core.ShapedArray(tuple(alloc.tensor_shape), mybir.dt.np(alloc.dtype))
            )
    n_params = len(in_names)
    n_outs = len(out_avals)
    all_names = tuple(in_names + out_names)
    if partition_name is not None:
        all_names = all_names + (partition_name,)
    donate = tuple(range(n_params, n_params + n_outs))

    def _body(*args):
        operands = list(args)
        if partition_name is not None:
            operands.append(bass2jax.partition_id_tensor())
        outs = bass2jax._bass_exec_p.bind(
            *operands,
            out_avals=tuple(out_avals),
            in_names=all_names,
            out_names=tuple(out_names),
            lowering_input_output_aliases=(),
            sim_require_finite=True,
            sim_require_nnan=True,
            nc=nc,
        )
        return tuple(outs)

    # unique per-runner name: the remote compile cache appears to key on the
    # jit module name, so distinct programs must not both be "jit__body"
    import hashlib

    _body.__name__ = "_body_" + hashlib.sha256(nc.to_json_bytes()).hexdigest()[:10]
    _body.__qualname__ = _body.__name__

    devices = jax.devices()[:n_cores]
    mesh = Mesh(np.asarray(devices), ("core",))
    in_specs = (PartitionSpec("core"),) * (n_params + n_outs)
    out_specs = (PartitionSpec("core"),) * n_outs
    sharded = jax.jit(
        shard_map(_body, mesh=mesh, in_specs=in_specs, out_specs=out_specs, check_rep=False),
        donate_argnums=donate,
        keep_unused=True,
    )

    def _collect(out_arrs):
        return [
            {
                k: np.asarray(out_arrs[i]).reshape(n_cores, *out_avals[i].shape)[c]
                for i, k in enumerate(out_names)
            }
            for c in range(n_cores)
        ]

    def run(in_maps):
        concat_in = [
            np.concatenate([np.asarray(in_maps[c][k]) for c in range(n_cores)], axis=0)
            for k in in_names
        ]
        concat_zeros = [
            np.zeros((n_cores * a.shape[0], *a.shape[1:]), a.dtype) for a in out_avals
        ]
        return _collect(sharded(*concat_in, *concat_zeros))

    def device_prep(in_maps):
        from jax.sharding import NamedSharding
        sh = NamedSharding(mesh, PartitionSpec("core"))
        return [
            jax.device_put(
                np.concatenate([np.asarray(in_maps[c][k]) for c in range(n_cores)], axis=0), sh
            )
            for k in in_names
        ]

    def device_call(dev_args, block=True):
        concat_zeros = [
            np.zeros((n_cores * a.shape[0], *a.shape[1:]), a.dtype) for a in out_avals
        ]
        out_arrs = sharded(*dev_args, *concat_zeros)
        if block:
            jax.block_until_ready(out_arrs)
        return out_arrs

    run.device_prep = device_prep
    run.device_call = device_call
    return run


def kernel(preds_S, preds_T, W_align, b_align, W_gen1, b_gen1, W_gen2, b_gen2):
    global LAST_RESULTS
    preds_S = np.asarray(preds_S, np.float32)
    preds_T = np.asarray(preds_T, np.float32)
    in_maps = _prep_inputs_v2(
        preds_S, preds_T,
        np.asarray(W_align, np.float32), np.asarray(b_align, np.float32),
        np.asarray(W_gen1, np.float32), np.asarray(b_gen1, np.float32),
        np.asarray(W_gen2, np.float32), np.asarray(b_gen2, np.float32),
    )
    for m in in_maps:
        m["vtag"] = np.zeros((1, 1), np.float32)
    if "run" not in _NC_CACHE:
        _NC_CACHE["run"] = _make_runner(_build_v2(), N_CORES)
    results = _NC_CACHE["run"](in_maps)
    LAST_RESULTS = results
    return np.float32(results[0]["loss"][0, 0])

